# revision 25
# baseline (speedup 1.0000x reference)
"""Trainium2 Bass kernel for nn_AffineCoupling (WaveGlow-style WN coupling).

Sharding: data-parallel over batch — B=8 samples, one per NeuronCore. All
convs are per-sample so no cross-core communication is needed.

Per-core plan (T=4096, chunked into 8 x 512 columns):
  - x (residual, 256ch) lives in SBUF as [128, 2*(128+4096+128)] fp32 with
    zero pads so dilated-conv taps are plain shifted column reads.
  - per layer: a = in_conv(x) (K=3 dilated, 6 matmuls/chunk/coutblk)
    + cond matmul (K=81: context padded with a ones-row so the combined
    in_b+cond_b bias folds into the weight row) -> PSUM [128, 2048];
    acts = tanh(a[:256]) * sigmoid(a[256:]) (ACT+DVE); res_skip 1x1 conv
    (2 matmuls/chunk/coutblk) -> PSUM; x/out updates via DVE
    scalar_tensor_tensor (fuses the rs bias add).
  - end conv (K=256 -> 8ch, channels reordered to [log_s, b]) + ACT bias,
    exp via ACT, coupling math via DVE, outputs DMA'd out per chunk.

Matmuls run as float32r (same bits as fp32, ~11-bit-mantissa PE path at full
bf16-rate throughput) — all matmul-feeding tensors are declared float32r
end-to-end so no cast copies exist anywhere. Measured ~0.57 ms/core on HW
with max relative error ~4e-4 vs the fp32 reference.
"""

import copy

import numpy as np

import concourse.bass as bass
import concourse.mybir as mybir
import concourse.tile as tile
from concourse import bass_utils
from concourse.vector_clock import ScopedClock

F32 = mybir.dt.float32
F32R = mybir.dt.float32r
AF = mybir.ActivationFunctionType
ALU = mybir.AluOpType

N_IN = 4
N_CTX = 80
L = 8
C = 256
DIL = [1, 2, 4, 8, 16, 32, 64, 128]
B, T = 8, 4096
TC = 512
NCHUNK = T // TC
PAD = 128
XW = PAD + T + PAD  # per cin-block padded width

MM_DT = mybir.dt.float32r  # matmul operand dtype tag


# ---------------------------------------------------------------------------
# Workarounds for the walrus build in this environment: it rejects any
# instruction carrying more than one sync-wait. Split extra waits onto
# single-wait carrier instructions placed just before the owner (same
# engine, so engine program order preserves semantics).
# ---------------------------------------------------------------------------


def _patched_drain_and_barrier(self, tick_clock, wait_clock):
    nc = self.nc
    tmp = nc.sync.nop(nofuse=True)
    wait_clock.add_sem_waits(tmp.ins, ScopedClock({None: tick_clock.global_clock}))
    si = tmp.ins.sync_info
    waits = list(si.on_wait) if si is not None else []
    if waits:
        si.on_wait = [waits[0]]
        for w in waits[1:]:
            n = nc.sync.nop(nofuse=True)
            n.ins.sync_info = mybir.SyncInfo(on_wait=[w], on_update=[])
    nc.sync.drain()
    nc.all_engine_barrier()
    popped = nc._tile_sem_poison_stack.pop()
    assert popped is self._sem_poison
    nc.clear_and_free_semaphores(list(self.sems.allocated().values()))
    nc.all_engine_barrier()


tile.TileContext._drain_and_barrier = _patched_drain_and_barrier


def split_multi_waits(nc):
    template = None
    for f in nc.m.functions:
        for bb in f.blocks:
            for inst in bb.instructions:
                if type(inst).__name__ == "InstEventSemaphore":
                    template = inst
                    break
            if template is not None:
                break
        if template is not None:
            break
    assert template is not None
    ctr = 0
    for f in nc.m.functions:
        for bb in f.blocks:
            insts = bb.instructions
            if not any(
                i.sync_info is not None and len(i.sync_info.on_wait) > 1
                for i in insts
            ):
                continue
            new = []
            for inst in insts:
                si = inst.sync_info
                if si is not None and len(si.on_wait) > 1:
                    waits = list(si.on_wait)
                    for w in waits[:-1]:
                        c = copy.copy(template)
                        c.name = f"waitsplit-{ctr}"
                        ctr += 1
                        c.engine = inst.engine
                        c.sync_info = mybir.SyncInfo(on_wait=[w], on_update=[])
                        new.append(c)
                    si.on_wait = [waits[-1]]
                new.append(inst)
            bb.instructions = new
    return ctr


# ---------------------------------------------------------------------------
# Program builder
# ---------------------------------------------------------------------------


def build_program():
    nc = bass.Bass("TRN2", target_bir_lowering=False, debug=False, num_devices=B)

    d_f0 = nc.dram_tensor("f0", [N_IN, T], F32R, kind="ExternalInput").ap()
    d_f1 = nc.dram_tensor("f1", [N_IN, T], F32, kind="ExternalInput").ap()
    d_ctx = nc.dram_tensor("ctxp", [N_CTX + 1, T], F32R, kind="ExternalInput").ap()
    d_win = nc.dram_tensor("w_in", [L, 128, 3072], F32R, kind="ExternalInput").ap()
    d_wrs = nc.dram_tensor("w_rs", [L, 128, 1024], F32R, kind="ExternalInput").ap()
    d_wcond = nc.dram_tensor("w_cond", [N_CTX + 1, 4096], F32R, kind="ExternalInput").ap()
    d_wstart = nc.dram_tensor("w_start", [N_IN, 256], F32R, kind="ExternalInput").ap()
    d_wend = nc.dram_tensor("w_end", [128, 16], F32R, kind="ExternalInput").ap()
    d_bend = nc.dram_tensor("b_end", [8, 1], F32, kind="ExternalInput").ap()
    d_rb = nc.dram_tensor("r_bias", [128, 4 * L], F32, kind="ExternalInput").ap()
    d_sb = nc.dram_tensor("s_bias", [128, 2], F32, kind="ExternalInput").ap()
    d_zeros = nc.dram_tensor("zeros", [128, 2 * PAD], F32R, kind="ExternalInput").ap()

    d_f1o = nc.dram_tensor("f1o", [N_IN, T], F32, kind="ExternalOutput").ap()
    d_logs = nc.dram_tensor("logs", [N_IN, T], F32, kind="ExternalOutput").ap()

    with tile.TileContext(nc) as tc:
        from contextlib import ExitStack

        ctx = ExitStack()
        with ctx:
            const = ctx.enter_context(tc.tile_pool(name="const", bufs=1))
            wpool = ctx.enter_context(tc.tile_pool(name="wpool", bufs=3))
            tspool = ctx.enter_context(tc.tile_pool(name="tspool", bufs=2))
            apool = ctx.enter_context(tc.tile_pool(name="apool", bufs=2))
            tmppool = ctx.enter_context(tc.tile_pool(name="tmppool", bufs=2))
            tailpool = ctx.enter_context(tc.tile_pool(name="tailpool", bufs=1))
            pspool = ctx.enter_context(tc.tile_pool(name="pspool", bufs=8, space="PSUM"))

            x = const.tile([128, 2 * XW], F32R, name="x")
            outacc = const.tile([128, 2 * T], F32R, name="outacc")
            ctxs = const.tile([N_CTX + 1, T], F32R, name="ctxs")
            f0s = const.tile([N_IN, T], F32R, name="f0s")
            conds = const.tile([N_CTX + 1, 4096], F32R, name="conds")
            starts = const.tile([N_IN, 256], F32R, name="starts")
            ends = const.tile([128, 16], F32R, name="ends")
            bendt = const.tile([8, 1], F32, name="bendt")
            rbt = const.tile([128, 4 * L], F32, name="rbt")
            sbt = const.tile([128, 2], F32, name="sbt")

            # critical-path loads on the SP queue (start conv inputs first)
            nc.sync.dma_start(starts[:], d_wstart[:])
            for j in range(NCHUNK):
                nc.sync.dma_start(
                    f0s[:, j * TC : (j + 1) * TC], d_f0[:, j * TC : (j + 1) * TC]
                )
            # layer-0 weights early on the SP queue so the first in-conv
            # matmuls don't wait behind anything else
            wint0 = wpool.tile([128, 3072], F32R, tag="win", name="wint0")
            nc.sync.dma_start(wint0[:], d_win[0])
            wrst0 = wpool.tile([128, 1024], F32R, tag="wrs", name="wrst0")
            nc.sync.dma_start(wrst0[:], d_wrs[0])
            nc.gpsimd.dma_start(sbt[:], d_sb[:])
            # bulk loads on the ACT HWDGE queue (start-conv copies were moved
            # to DVE so these descriptor writes don't block anything)
            nc.scalar.dma_start(rbt[:], d_rb[:])
            for j in range(NCHUNK):
                nc.scalar.dma_start(
                    ctxs[:, j * TC : (j + 1) * TC], d_ctx[:, j * TC : (j + 1) * TC]
                )
            for i in range(L):
                nc.scalar.dma_start(
                    conds[:, i * TC : (i + 1) * TC], d_wcond[:, i * TC : (i + 1) * TC]
                )
            nc.scalar.dma_start(ends[:], d_wend[:])
            nc.scalar.dma_start(bendt[:], d_bend[:])

            # zero the halo columns of x once; updates only touch the center
            # (DMA'd zeros — DVE memset can't write float32r)
            nc.gpsimd.dma_start(x[:, 0:PAD], d_zeros[:, 0:PAD])
            nc.gpsimd.dma_start(x[:, PAD + T : XW + PAD], d_zeros[:])
            nc.gpsimd.dma_start(x[:, XW + PAD + T : 2 * XW], d_zeros[:, 0:PAD])

            # ---- start conv: x = start_w @ f0 + start_b ----
            for j in range(NCHUNK):
                ps_s = [
                    pspool.tile([128, TC], F32, tag="ps", name=f"ps_s{c}")
                    for c in range(2)
                ]
                for c in range(2):
                    nc.tensor.matmul(
                        ps_s[c][:],
                        lhsT=starts[:, c * 128 : (c + 1) * 128],
                        rhs=f0s[:, j * TC : (j + 1) * TC],
                        start=True,
                        stop=True,
                    )
                for c in range(2):
                    nc.vector.tensor_scalar_add(
                        x[:, c * XW + PAD + j * TC : c * XW + PAD + (j + 1) * TC],
                        ps_s[c][:],
                        sbt[:, c : c + 1],
                    )

            # ---- WN layers ----
            # Software-pipelined: the gating/res-skip/update work for chunk j
            # is emitted AFTER chunk j+1's in-conv matmuls, so (a) the PE
            # stream never waits on the ACT->DVE gating chain, and (b) the
            # x update for chunk j lands after chunk j+1's k=0 tap has read
            # the pre-update tail (correctness of the dilated conv halo).
            def chunk_tail(st_):
                i, j, ps_a, wrst, last = (
                    st_["i"],
                    st_["j"],
                    st_["ps_a"],
                    st_["wrst"],
                    st_["last"],
                )
                tt = tspool.tile([128, 1024], F32, tag="tt", name="tt")
                st = tspool.tile([128, 1024], F32, tag="st", name="st")
                for c in range(2):
                    nc.scalar.activation(
                        tt[:, c * TC : (c + 1) * TC], ps_a[c][:], AF.Tanh
                    )
                    nc.scalar.activation(
                        st[:, c * TC : (c + 1) * TC], ps_a[2 + c][:], AF.Sigmoid
                    )
                actst = apool.tile([128, 1024], F32R, tag="acts", name="actst")
                nc.vector.tensor_mul(actst[:], tt[:], st[:])

                nco = 2 if last else 4
                ps_r = [
                    pspool.tile([128, TC], F32, tag="ps", name=f"ps_r{c2}")
                    for c2 in range(nco)
                ]
                for c2 in range(nco):
                    for p in range(2):
                        nc.tensor.matmul(
                            ps_r[c2][:],
                            lhsT=wrst[
                                :,
                                p * TC + c2 * 128 : p * TC + (c2 + 1) * 128,
                            ],
                            rhs=actst[:, p * TC : (p + 1) * TC],
                            start=(p == 0),
                            stop=(p == 1),
                        )
                if not last:
                    for c2 in range(2):
                        xs = x[
                            :, c2 * XW + PAD + j * TC : c2 * XW + PAD + (j + 1) * TC
                        ]
                        nc.vector.scalar_tensor_tensor(
                            xs,
                            ps_r[c2][:],
                            rbt[:, i * 4 + c2 : i * 4 + c2 + 1],
                            xs,
                            ALU.add,
                            ALU.add,
                        )
                    for c2 in range(2, 4):
                        os_ = outacc[
                            :, (c2 - 2) * T + j * TC : (c2 - 2) * T + (j + 1) * TC
                        ]
                        if i == 0:
                            nc.vector.tensor_scalar_add(
                                os_,
                                ps_r[c2][:],
                                rbt[:, i * 4 + c2 : i * 4 + c2 + 1],
                            )
                        else:
                            nc.vector.scalar_tensor_tensor(
                                os_,
                                ps_r[c2][:],
                                rbt[:, i * 4 + c2 : i * 4 + c2 + 1],
                                os_,
                                ALU.add,
                                ALU.add,
                            )
                else:
                    for c2 in range(2):
                        os_ = outacc[:, c2 * T + j * TC : c2 * T + (j + 1) * TC]
                        nc.vector.scalar_tensor_tensor(
                            os_,
                            ps_r[c2][:],
                            rbt[:, i * 4 + c2 : i * 4 + c2 + 1],
                            os_,
                            ALU.add,
                            ALU.add,
                        )

            # ---- end conv + coupling (emitted per-chunk, interleaved) ----
            def end_chunk(j):
                ps_e = pspool.tile([8, TC], F32, tag="ps", name="ps_e")
                for p in range(2):
                    nc.tensor.matmul(
                        ps_e[:],
                        lhsT=ends[:, p * 8 : (p + 1) * 8],
                        rhs=outacc[:, p * T + j * TC : p * T + (j + 1) * TC],
                        start=(p == 0),
                        stop=(p == 1),
                    )
                esb = tailpool.tile([8, TC], F32, tag="esb", name="esb")
                nc.scalar.activation(esb[:], ps_e[:], AF.Identity, bias=bendt[:])
                nc.sync.dma_start(d_logs[:, j * TC : (j + 1) * TC], esb[0:N_IN, :])
                expt = tailpool.tile([N_IN, TC], F32, tag="expt", name="expt")
                nc.scalar.activation(expt[:], esb[0:N_IN, :], AF.Exp)
                bsh = tailpool.tile([N_IN, TC], F32, tag="bsh", name="bsh")
                nc.sync.dma_start(bsh[:], esb[N_IN : 2 * N_IN, :])
                f1c = tailpool.tile([N_IN, TC], F32, tag="f1c", name="f1c")
                nc.sync.dma_start(f1c[:], d_f1[:, j * TC : (j + 1) * TC])
                f1oc = tailpool.tile([N_IN, TC], F32, tag="f1oc", name="f1oc")
                nc.vector.tensor_mul(f1oc[:], expt[:], f1c[:])
                nc.vector.tensor_add(f1oc[:], f1oc[:], bsh[:])
                nc.sync.dma_start(d_f1o[:, j * TC : (j + 1) * TC], f1oc[:])


            prev = None  # pipeline state carried across chunks AND layers
            for i in range(L):
                d = DIL[i]
                last = i == L - 1
                if i == 0:
                    wint, wrst = wint0, wrst0
                else:
                    wint = wpool.tile([128, 3072], F32R, tag="win", name="wint")
                    nc.sync.dma_start(wint[:], d_win[i])
                    wrst = wpool.tile([128, 1024], F32R, tag="wrs", name="wrst")
                    nc.sync.dma_start(wrst[:], d_wrs[i])

                for j in range(NCHUNK):
                    ps_a = [
                        pspool.tile([128, TC], F32, tag="ps", name=f"ps_a{c}")
                        for c in range(4)
                    ]
                    for c in range(4):
                        po = ps_a[c][:]
                        for k in range(3):
                            off = PAD + j * TC + (k - 1) * d
                            for p in range(2):
                                nc.tensor.matmul(
                                    po,
                                    lhsT=wint[
                                        :,
                                        (k * 2 + p) * TC
                                        + c * 128 : (k * 2 + p) * TC
                                        + (c + 1) * 128,
                                    ],
                                    rhs=x[:, p * XW + off : p * XW + off + TC],
                                    start=(k == 0 and p == 0),
                                    stop=False,
                                )
                        nc.tensor.matmul(
                            po,
                            lhsT=conds[:, i * TC + c * 128 : i * TC + (c + 1) * 128],
                            rhs=ctxs[:, j * TC : (j + 1) * TC],
                            start=False,
                            stop=True,
                        )
                    if prev is not None:
                        chunk_tail(prev)
                        if prev["last"]:
                            end_chunk(prev["j"])
                    prev = {"i": i, "j": j, "ps_a": ps_a, "wrst": wrst, "last": last}
            chunk_tail(prev)
            end_chunk(prev["j"])
            prev = None

    split_multi_waits(nc)
    return nc


# ---------------------------------------------------------------------------
# Host-side weight/layout prep
# ---------------------------------------------------------------------------


def prep_shared(inputs):
    f32 = np.float32
    in_w = np.ascontiguousarray(inputs["in_w"], f32)  # [8, 512, 256, 3]
    w_in = np.ascontiguousarray(
        in_w.reshape(L, 2 * C, 2, 128, 3).transpose(0, 3, 4, 2, 1).reshape(L, 128, 3072)
    )
    rs_w = np.ascontiguousarray(inputs["rs_w"], f32)[..., 0]  # [8, 512, 256]
    w_rs = np.ascontiguousarray(
        rs_w.reshape(L, 2 * C, 2, 128).transpose(0, 3, 2, 1).reshape(L, 128, 1024)
    )
    cond_w = np.ascontiguousarray(inputs["cond_w"], f32)[..., 0]  # [4096, 80]
    ab = inputs["in_b"].reshape(-1) + inputs["cond_b"]  # [4096]
    w_cond = np.concatenate([cond_w.T, ab[None, :]], axis=0).astype(f32)  # [81, 4096]
    w_start = np.ascontiguousarray(inputs["start_w"][..., 0].T, f32)  # [4, 256]
    perm = [N_IN + k for k in range(N_IN)] + list(range(N_IN))  # [log_s..., b...]
    ew = inputs["end_w"][..., 0][perm]  # [8, 256]
    w_end = np.ascontiguousarray(
        ew.T.reshape(2, 128, 8).transpose(1, 0, 2).reshape(128, 16), f32
    )
    b_end = np.ascontiguousarray(inputs["end_b"][perm][:, None], f32)  # [8, 1]
    r_bias = np.ascontiguousarray(
        inputs["rs_b"].reshape(L, 4, 128).transpose(2, 0, 1).reshape(128, 4 * L), f32
    )
    s_bias = np.ascontiguousarray(inputs["start_b"].reshape(2, 128).T, f32)  # [128, 2]
    return {
        "w_in": w_in,
        "w_rs": w_rs,
        "w_cond": w_cond,
        "w_start": w_start,
        "w_end": w_end,
        "b_end": b_end,
        "r_bias": r_bias,
        "s_bias": s_bias,
    }


def _ensure_ntff_hook():
    """Register the axon NTFF profiling hook if the image's antenv lacks it."""
    import sys
    import types

    try:
        import antenv.axon_hooks  # noqa: F401

        return
    except ImportError:
        pass
    mod = types.ModuleType("antenv.axon_hooks")
    holder = [None]
    mod.set_axon_ntff_profile_hook = lambda h: holder.__setitem__(0, h)
    mod.get_axon_ntff_profile_hook = lambda: holder[0]
    sys.modules["antenv.axon_hooks"] = mod
    try:
        from trn_agent_boot.trn_boot import _ntff_profile_via_ctypes

        mod.set_axon_ntff_profile_hook(
            _ntff_profile_via_ctypes("/opt/axon/libaxon_pjrt.so")
        )
    except Exception:
        pass


_NC = None


def _get_program():
    global _NC
    if _NC is None:
        _NC = build_program()
    return _NC


LAST_RESULTS = None


def kernel(forecast, context, start_w, start_b, cond_w, cond_b,
           in_w, in_b, rs_w, rs_b, end_w, end_b, _trace=False):
    global LAST_RESULTS
    inputs = dict(
        forecast=np.asarray(forecast), context=np.asarray(context),
        start_w=np.asarray(start_w), start_b=np.asarray(start_b),
        cond_w=np.asarray(cond_w), cond_b=np.asarray(cond_b),
        in_w=np.asarray(in_w), in_b=np.asarray(in_b),
        rs_w=np.asarray(rs_w), rs_b=np.asarray(rs_b),
        end_w=np.asarray(end_w), end_b=np.asarray(end_b),
    )
    shared = prep_shared(inputs)
    shared["zeros"] = np.zeros((128, 2 * PAD), np.float32)
    forecast = np.ascontiguousarray(inputs["forecast"], np.float32)
    context = np.ascontiguousarray(inputs["context"], np.float32)
    ones_row = np.ones((1, T), np.float32)

    in_maps = []
    for b in range(B):
        m = dict(shared)
        m["f0"] = np.ascontiguousarray(forecast[b, :N_IN])
        m["f1"] = np.ascontiguousarray(forecast[b, N_IN:])
        m["ctxp"] = np.ascontiguousarray(
            np.concatenate([context[b], ones_row], axis=0)
        )
        in_maps.append(m)

    if _trace:
        _ensure_ntff_hook()
    nc = _get_program()
    try:
        res = bass_utils.run_bass_kernel_spmd(
            nc, in_maps, core_ids=list(range(B)), trace=_trace
        )
    except Exception:
        # transient device errors (e.g. NRT_EXEC_UNIT_UNRECOVERABLE after a
        # prior crashed run) usually clear on retry with a fresh NRT session
        import time

        time.sleep(2.0)
        res = bass_utils.run_bass_kernel_spmd(
            nc, in_maps, core_ids=list(range(B)), trace=_trace
        )
    LAST_RESULTS = res

    f1o = np.stack([res.results[b]["f1o"] for b in range(B)])  # [8, 4, T]
    logs = np.stack([res.results[b]["logs"] for b in range(B)])  # [8, 4, T]
    out_full = np.concatenate([forecast[:, :N_IN], f1o], axis=1)  # [8, 8, T]
    return out_full, logs


# revision 26
# speedup vs baseline: 1.0035x; 1.0035x over previous
"""Trainium2 Bass kernel for nn_AffineCoupling (WaveGlow-style WN coupling).

Sharding: data-parallel over batch — B=8 samples, one per NeuronCore. All
convs are per-sample so no cross-core communication is needed.

Per-core plan (T=4096, chunked into 8 x 512 columns):
  - x (residual, 256ch) lives in SBUF as [128, 2*(128+4096+128)] fp32 with
    zero pads so dilated-conv taps are plain shifted column reads.
  - per layer: a = in_conv(x) (K=3 dilated, 6 matmuls/chunk/coutblk)
    + cond matmul (K=81: context padded with a ones-row so the combined
    in_b+cond_b bias folds into the weight row) -> PSUM [128, 2048];
    acts = tanh(a[:256]) * sigmoid(a[256:]) (ACT+DVE); res_skip 1x1 conv
    (2 matmuls/chunk/coutblk) -> PSUM; x/out updates via DVE
    scalar_tensor_tensor (fuses the rs bias add).
  - end conv (K=256 -> 8ch, channels reordered to [log_s, b]) + ACT bias,
    exp via ACT, coupling math via DVE, outputs DMA'd out per chunk.

Matmuls run as float32r (same bits as fp32, ~11-bit-mantissa PE path at full
bf16-rate throughput) — all matmul-feeding tensors are declared float32r
end-to-end so no cast copies exist anywhere. Measured ~0.57 ms/core on HW
with max relative error ~4e-4 vs the fp32 reference.
"""

import copy

import numpy as np

import concourse.bass as bass
import concourse.mybir as mybir
import concourse.tile as tile
from concourse import bass_utils
from concourse.vector_clock import ScopedClock

F32 = mybir.dt.float32
F32R = mybir.dt.float32r
AF = mybir.ActivationFunctionType
ALU = mybir.AluOpType

N_IN = 4
N_CTX = 80
L = 8
C = 256
DIL = [1, 2, 4, 8, 16, 32, 64, 128]
B, T = 8, 4096
TC = 512
NCHUNK = T // TC
PAD = 128
XW = PAD + T + PAD  # per cin-block padded width

MM_DT = mybir.dt.float32r  # matmul operand dtype tag


# ---------------------------------------------------------------------------
# Workarounds for the walrus build in this environment: it rejects any
# instruction carrying more than one sync-wait. Split extra waits onto
# single-wait carrier instructions placed just before the owner (same
# engine, so engine program order preserves semantics).
# ---------------------------------------------------------------------------


def _patched_drain_and_barrier(self, tick_clock, wait_clock):
    nc = self.nc
    tmp = nc.sync.nop(nofuse=True)
    wait_clock.add_sem_waits(tmp.ins, ScopedClock({None: tick_clock.global_clock}))
    si = tmp.ins.sync_info
    waits = list(si.on_wait) if si is not None else []
    if waits:
        si.on_wait = [waits[0]]
        for w in waits[1:]:
            n = nc.sync.nop(nofuse=True)
            n.ins.sync_info = mybir.SyncInfo(on_wait=[w], on_update=[])
    nc.sync.drain()
    nc.all_engine_barrier()
    popped = nc._tile_sem_poison_stack.pop()
    assert popped is self._sem_poison
    nc.clear_and_free_semaphores(list(self.sems.allocated().values()))
    nc.all_engine_barrier()


tile.TileContext._drain_and_barrier = _patched_drain_and_barrier


def split_multi_waits(nc):
    template = None
    for f in nc.m.functions:
        for bb in f.blocks:
            for inst in bb.instructions:
                if type(inst).__name__ == "InstEventSemaphore":
                    template = inst
                    break
            if template is not None:
                break
        if template is not None:
            break
    assert template is not None
    ctr = 0
    for f in nc.m.functions:
        for bb in f.blocks:
            insts = bb.instructions
            if not any(
                i.sync_info is not None and len(i.sync_info.on_wait) > 1
                for i in insts
            ):
                continue
            new = []
            for inst in insts:
                si = inst.sync_info
                if si is not None and len(si.on_wait) > 1:
                    waits = list(si.on_wait)
                    for w in waits[:-1]:
                        c = copy.copy(template)
                        c.name = f"waitsplit-{ctr}"
                        ctr += 1
                        c.engine = inst.engine
                        c.sync_info = mybir.SyncInfo(on_wait=[w], on_update=[])
                        new.append(c)
                    si.on_wait = [waits[-1]]
                new.append(inst)
            bb.instructions = new
    return ctr


# ---------------------------------------------------------------------------
# Program builder
# ---------------------------------------------------------------------------


def build_program():
    nc = bass.Bass("TRN2", target_bir_lowering=False, debug=False, num_devices=B)

    d_f0 = nc.dram_tensor("f0", [N_IN, T], F32R, kind="ExternalInput").ap()
    d_f1 = nc.dram_tensor("f1", [N_IN, T], F32, kind="ExternalInput").ap()
    d_ctx = nc.dram_tensor("ctxp", [N_CTX + 1, T], F32R, kind="ExternalInput").ap()
    d_win = nc.dram_tensor("w_in", [L, 128, 3072], F32R, kind="ExternalInput").ap()
    d_wrs = nc.dram_tensor("w_rs", [L, 128, 1024], F32R, kind="ExternalInput").ap()
    d_wcond = nc.dram_tensor("w_cond", [N_CTX + 1, 4096], F32R, kind="ExternalInput").ap()
    d_wstart = nc.dram_tensor("w_start", [N_IN, 256], F32R, kind="ExternalInput").ap()
    d_wend = nc.dram_tensor("w_end", [128, 16], F32R, kind="ExternalInput").ap()
    d_bend = nc.dram_tensor("b_end", [8, 1], F32, kind="ExternalInput").ap()
    d_rb = nc.dram_tensor("r_bias", [128, 4 * L], F32, kind="ExternalInput").ap()
    d_sb = nc.dram_tensor("s_bias", [128, 2], F32, kind="ExternalInput").ap()
    d_zeros = nc.dram_tensor("zeros", [128, 2 * PAD], F32R, kind="ExternalInput").ap()

    d_f1o = nc.dram_tensor("f1o", [N_IN, T], F32, kind="ExternalOutput").ap()
    d_logs = nc.dram_tensor("logs", [N_IN, T], F32, kind="ExternalOutput").ap()

    with tile.TileContext(nc) as tc:
        from contextlib import ExitStack

        ctx = ExitStack()
        with ctx:
            const = ctx.enter_context(tc.tile_pool(name="const", bufs=1))
            wpool = ctx.enter_context(tc.tile_pool(name="wpool", bufs=3))
            tspool = ctx.enter_context(tc.tile_pool(name="tspool", bufs=2))
            apool = ctx.enter_context(tc.tile_pool(name="apool", bufs=2))
            tailpool = ctx.enter_context(tc.tile_pool(name="tailpool", bufs=1))
            pspool = ctx.enter_context(tc.tile_pool(name="pspool", bufs=8, space="PSUM"))

            x = const.tile([128, 2 * XW], F32R, name="x")
            outacc = const.tile([128, 2 * T], F32R, name="outacc")
            ctxs = const.tile([N_CTX + 1, T], F32R, name="ctxs")
            f0s = const.tile([N_IN, T], F32R, name="f0s")
            conds = const.tile([N_CTX + 1, 4096], F32R, name="conds")
            starts = const.tile([N_IN, 256], F32R, name="starts")
            ends = const.tile([128, 16], F32R, name="ends")
            bendt = const.tile([8, 1], F32, name="bendt")
            rbt = const.tile([128, 4 * L], F32, name="rbt")
            sbt = const.tile([128, 2], F32, name="sbt")

            # critical-path loads on the SP queue (start conv inputs first)
            nc.sync.dma_start(starts[:], d_wstart[:])
            for j in range(NCHUNK):
                nc.sync.dma_start(
                    f0s[:, j * TC : (j + 1) * TC], d_f0[:, j * TC : (j + 1) * TC]
                )
            # layer-0 weights early on the SP queue so the first in-conv
            # matmuls don't wait behind anything else
            wint0 = wpool.tile([128, 3072], F32R, tag="win", name="wint0")
            nc.sync.dma_start(wint0[:], d_win[0])
            wrst0 = wpool.tile([128, 1024], F32R, tag="wrs", name="wrst0")
            nc.sync.dma_start(wrst0[:], d_wrs[0])
            nc.gpsimd.dma_start(sbt[:], d_sb[:])
            # bulk loads on the ACT HWDGE queue (start-conv copies were moved
            # to DVE so these descriptor writes don't block anything)
            nc.scalar.dma_start(rbt[:], d_rb[:])
            for j in range(NCHUNK):
                nc.scalar.dma_start(
                    ctxs[:, j * TC : (j + 1) * TC], d_ctx[:, j * TC : (j + 1) * TC]
                )
            for i in range(L):
                nc.scalar.dma_start(
                    conds[:, i * TC : (i + 1) * TC], d_wcond[:, i * TC : (i + 1) * TC]
                )
            nc.scalar.dma_start(ends[:], d_wend[:])
            nc.scalar.dma_start(bendt[:], d_bend[:])

            # zero the halo columns of x once; updates only touch the center
            # (DMA'd zeros — DVE memset can't write float32r)
            nc.gpsimd.dma_start(x[:, 0:PAD], d_zeros[:, 0:PAD])
            nc.gpsimd.dma_start(x[:, PAD + T : XW + PAD], d_zeros[:])
            nc.gpsimd.dma_start(x[:, XW + PAD + T : 2 * XW], d_zeros[:, 0:PAD])

            # ---- start conv: x = start_w @ f0 + start_b ----
            for j in range(NCHUNK):
                ps_s = [
                    pspool.tile([128, TC], F32, tag="ps", name=f"ps_s{c}")
                    for c in range(2)
                ]
                for c in range(2):
                    nc.tensor.matmul(
                        ps_s[c][:],
                        lhsT=starts[:, c * 128 : (c + 1) * 128],
                        rhs=f0s[:, j * TC : (j + 1) * TC],
                        start=True,
                        stop=True,
                    )
                for c in range(2):
                    nc.vector.tensor_scalar_add(
                        x[:, c * XW + PAD + j * TC : c * XW + PAD + (j + 1) * TC],
                        ps_s[c][:],
                        sbt[:, c : c + 1],
                    )

            # ---- WN layers ----
            # Software-pipelined: the gating/res-skip/update work for chunk j
            # is emitted AFTER chunk j+1's in-conv matmuls, so (a) the PE
            # stream never waits on the ACT->DVE gating chain, and (b) the
            # x update for chunk j lands after chunk j+1's k=0 tap has read
            # the pre-update tail (correctness of the dilated conv halo).
            def chunk_tail(st_):
                i, j, ps_a, wrst, last = (
                    st_["i"],
                    st_["j"],
                    st_["ps_a"],
                    st_["wrst"],
                    st_["last"],
                )
                tt = tspool.tile([128, 1024], F32, tag="tt", name="tt")
                st = tspool.tile([128, 1024], F32, tag="st", name="st")
                for c in range(2):
                    nc.scalar.activation(
                        tt[:, c * TC : (c + 1) * TC], ps_a[c][:], AF.Tanh
                    )
                    nc.scalar.activation(
                        st[:, c * TC : (c + 1) * TC], ps_a[2 + c][:], AF.Sigmoid
                    )
                actst = apool.tile([128, 1024], F32R, tag="acts", name="actst")
                nc.vector.tensor_mul(actst[:], tt[:], st[:])

                nco = 2 if last else 4
                ps_r = [
                    pspool.tile([128, TC], F32, tag="ps", name=f"ps_r{c2}")
                    for c2 in range(nco)
                ]
                for c2 in range(nco):
                    for p in range(2):
                        nc.tensor.matmul(
                            ps_r[c2][:],
                            lhsT=wrst[
                                :,
                                p * TC + c2 * 128 : p * TC + (c2 + 1) * 128,
                            ],
                            rhs=actst[:, p * TC : (p + 1) * TC],
                            start=(p == 0),
                            stop=(p == 1),
                        )
                if not last:
                    for c2 in range(2):
                        xs = x[
                            :, c2 * XW + PAD + j * TC : c2 * XW + PAD + (j + 1) * TC
                        ]
                        nc.vector.scalar_tensor_tensor(
                            xs,
                            ps_r[c2][:],
                            rbt[:, i * 4 + c2 : i * 4 + c2 + 1],
                            xs,
                            ALU.add,
                            ALU.add,
                        )
                    for c2 in range(2, 4):
                        os_ = outacc[
                            :, (c2 - 2) * T + j * TC : (c2 - 2) * T + (j + 1) * TC
                        ]
                        if i == 0:
                            nc.vector.tensor_scalar_add(
                                os_,
                                ps_r[c2][:],
                                rbt[:, i * 4 + c2 : i * 4 + c2 + 1],
                            )
                        else:
                            nc.vector.scalar_tensor_tensor(
                                os_,
                                ps_r[c2][:],
                                rbt[:, i * 4 + c2 : i * 4 + c2 + 1],
                                os_,
                                ALU.add,
                                ALU.add,
                            )
                else:
                    for c2 in range(2):
                        os_ = outacc[:, c2 * T + j * TC : c2 * T + (j + 1) * TC]
                        nc.vector.scalar_tensor_tensor(
                            os_,
                            ps_r[c2][:],
                            rbt[:, i * 4 + c2 : i * 4 + c2 + 1],
                            os_,
                            ALU.add,
                            ALU.add,
                        )

            # ---- end conv + coupling (emitted per-chunk, interleaved) ----
            def end_chunk(j):
                ps_e = pspool.tile([8, TC], F32, tag="ps", name="ps_e")
                for p in range(2):
                    nc.tensor.matmul(
                        ps_e[:],
                        lhsT=ends[:, p * 8 : (p + 1) * 8],
                        rhs=outacc[:, p * T + j * TC : p * T + (j + 1) * TC],
                        start=(p == 0),
                        stop=(p == 1),
                    )
                esb = tailpool.tile([8, TC], F32, tag="esb", name="esb")
                nc.scalar.activation(esb[:], ps_e[:], AF.Identity, bias=bendt[:])
                nc.sync.dma_start(d_logs[:, j * TC : (j + 1) * TC], esb[0:N_IN, :])
                expt = tailpool.tile([N_IN, TC], F32, tag="expt", name="expt")
                nc.scalar.activation(expt[:], esb[0:N_IN, :], AF.Exp)
                bsh = tailpool.tile([N_IN, TC], F32, tag="bsh", name="bsh")
                nc.sync.dma_start(bsh[:], esb[N_IN : 2 * N_IN, :])
                f1c = tailpool.tile([N_IN, TC], F32, tag="f1c", name="f1c")
                nc.sync.dma_start(f1c[:], d_f1[:, j * TC : (j + 1) * TC])
                f1oc = tailpool.tile([N_IN, TC], F32, tag="f1oc", name="f1oc")
                nc.vector.tensor_mul(f1oc[:], expt[:], f1c[:])
                nc.vector.tensor_add(f1oc[:], f1oc[:], bsh[:])
                nc.sync.dma_start(d_f1o[:, j * TC : (j + 1) * TC], f1oc[:])


            prev = None  # pipeline state carried across chunks AND layers
            for i in range(L):
                d = DIL[i]
                last = i == L - 1
                if i == 0:
                    wint, wrst = wint0, wrst0
                else:
                    wint = wpool.tile([128, 3072], F32R, tag="win", name="wint")
                    nc.sync.dma_start(wint[:], d_win[i])
                    wrst = wpool.tile([128, 1024], F32R, tag="wrs", name="wrst")
                    nc.sync.dma_start(wrst[:], d_wrs[i])

                for j in range(NCHUNK):
                    ps_a = [
                        pspool.tile([128, TC], F32, tag="ps", name=f"ps_a{c}")
                        for c in range(4)
                    ]
                    for c in range(4):
                        po = ps_a[c][:]
                        for k in range(3):
                            off = PAD + j * TC + (k - 1) * d
                            for p in range(2):
                                nc.tensor.matmul(
                                    po,
                                    lhsT=wint[
                                        :,
                                        (k * 2 + p) * TC
                                        + c * 128 : (k * 2 + p) * TC
                                        + (c + 1) * 128,
                                    ],
                                    rhs=x[:, p * XW + off : p * XW + off + TC],
                                    start=(k == 0 and p == 0),
                                    stop=False,
                                )
                        nc.tensor.matmul(
                            po,
                            lhsT=conds[:, i * TC + c * 128 : i * TC + (c + 1) * 128],
                            rhs=ctxs[:, j * TC : (j + 1) * TC],
                            start=False,
                            stop=True,
                        )
                    if prev is not None:
                        chunk_tail(prev)
                        if prev["last"]:
                            end_chunk(prev["j"])
                    prev = {"i": i, "j": j, "ps_a": ps_a, "wrst": wrst, "last": last}
            chunk_tail(prev)
            end_chunk(prev["j"])
            prev = None

    split_multi_waits(nc)
    return nc


# ---------------------------------------------------------------------------
# Host-side weight/layout prep
# ---------------------------------------------------------------------------


def prep_shared(inputs):
    f32 = np.float32
    in_w = np.ascontiguousarray(inputs["in_w"], f32)  # [8, 512, 256, 3]
    w_in = np.ascontiguousarray(
        in_w.reshape(L, 2 * C, 2, 128, 3).transpose(0, 3, 4, 2, 1).reshape(L, 128, 3072)
    )
    rs_w = np.ascontiguousarray(inputs["rs_w"], f32)[..., 0]  # [8, 512, 256]
    w_rs = np.ascontiguousarray(
        rs_w.reshape(L, 2 * C, 2, 128).transpose(0, 3, 2, 1).reshape(L, 128, 1024)
    )
    cond_w = np.ascontiguousarray(inputs["cond_w"], f32)[..., 0]  # [4096, 80]
    ab = inputs["in_b"].reshape(-1) + inputs["cond_b"]  # [4096]
    w_cond = np.concatenate([cond_w.T, ab[None, :]], axis=0).astype(f32)  # [81, 4096]
    w_start = np.ascontiguousarray(inputs["start_w"][..., 0].T, f32)  # [4, 256]
    perm = [N_IN + k for k in range(N_IN)] + list(range(N_IN))  # [log_s..., b...]
    ew = inputs["end_w"][..., 0][perm]  # [8, 256]
    w_end = np.ascontiguousarray(
        ew.T.reshape(2, 128, 8).transpose(1, 0, 2).reshape(128, 16), f32
    )
    b_end = np.ascontiguousarray(inputs["end_b"][perm][:, None], f32)  # [8, 1]
    r_bias = np.ascontiguousarray(
        inputs["rs_b"].reshape(L, 4, 128).transpose(2, 0, 1).reshape(128, 4 * L), f32
    )
    s_bias = np.ascontiguousarray(inputs["start_b"].reshape(2, 128).T, f32)  # [128, 2]
    return {
        "w_in": w_in,
        "w_rs": w_rs,
        "w_cond": w_cond,
        "w_start": w_start,
        "w_end": w_end,
        "b_end": b_end,
        "r_bias": r_bias,
        "s_bias": s_bias,
    }


def _ensure_ntff_hook():
    """Register the axon NTFF profiling hook if the image's antenv lacks it."""
    import sys
    import types

    try:
        import antenv.axon_hooks  # noqa: F401

        return
    except ImportError:
        pass
    mod = types.ModuleType("antenv.axon_hooks")
    holder = [None]
    mod.set_axon_ntff_profile_hook = lambda h: holder.__setitem__(0, h)
    mod.get_axon_ntff_profile_hook = lambda: holder[0]
    sys.modules["antenv.axon_hooks"] = mod
    try:
        from trn_agent_boot.trn_boot import _ntff_profile_via_ctypes

        mod.set_axon_ntff_profile_hook(
            _ntff_profile_via_ctypes("/opt/axon/libaxon_pjrt.so")
        )
    except Exception:
        pass


_NC = None


def _get_program():
    global _NC
    if _NC is None:
        _NC = build_program()
    return _NC


LAST_RESULTS = None


def kernel(forecast, context, start_w, start_b, cond_w, cond_b,
           in_w, in_b, rs_w, rs_b, end_w, end_b, _trace=False):
    global LAST_RESULTS
    inputs = dict(
        forecast=np.asarray(forecast), context=np.asarray(context),
        start_w=np.asarray(start_w), start_b=np.asarray(start_b),
        cond_w=np.asarray(cond_w), cond_b=np.asarray(cond_b),
        in_w=np.asarray(in_w), in_b=np.asarray(in_b),
        rs_w=np.asarray(rs_w), rs_b=np.asarray(rs_b),
        end_w=np.asarray(end_w), end_b=np.asarray(end_b),
    )
    shared = prep_shared(inputs)
    shared["zeros"] = np.zeros((128, 2 * PAD), np.float32)
    forecast = np.ascontiguousarray(inputs["forecast"], np.float32)
    context = np.ascontiguousarray(inputs["context"], np.float32)
    ones_row = np.ones((1, T), np.float32)

    in_maps = []
    for b in range(B):
        m = dict(shared)
        m["f0"] = np.ascontiguousarray(forecast[b, :N_IN])
        m["f1"] = np.ascontiguousarray(forecast[b, N_IN:])
        m["ctxp"] = np.ascontiguousarray(
            np.concatenate([context[b], ones_row], axis=0)
        )
        in_maps.append(m)

    if _trace:
        _ensure_ntff_hook()
    nc = _get_program()
    try:
        res = bass_utils.run_bass_kernel_spmd(
            nc, in_maps, core_ids=list(range(B)), trace=_trace
        )
    except Exception:
        # transient device errors (e.g. NRT_EXEC_UNIT_UNRECOVERABLE after a
        # prior crashed run) usually clear on retry with a fresh NRT session
        import time

        time.sleep(2.0)
        res = bass_utils.run_bass_kernel_spmd(
            nc, in_maps, core_ids=list(range(B)), trace=_trace
        )
    LAST_RESULTS = res

    f1o = np.stack([res.results[b]["f1o"] for b in range(B)])  # [8, 4, T]
    logs = np.stack([res.results[b]["logs"] for b in range(B)])  # [8, 4, T]
    out_full = np.concatenate([forecast[:, :N_IN], f1o], axis=1)  # [8, 8, T]
    return out_full, logs


# revision 27
# speedup vs baseline: 1.0037x; 1.0002x over previous
"""Trainium2 Bass kernel for nn_AffineCoupling (WaveGlow-style WN coupling).

Sharding: data-parallel over batch — B=8 samples, one per NeuronCore. All
convs are per-sample so no cross-core communication is needed.

Per-core plan (T=4096, chunked into 8 x 512 columns):
  - x (residual, 256ch) lives in SBUF as [128, 2*(128+4096+128)] fp32 with
    zero pads so dilated-conv taps are plain shifted column reads.
  - per layer: a = in_conv(x) (K=3 dilated, 6 matmuls/chunk/coutblk)
    + cond matmul (K=81: context padded with a ones-row so the combined
    in_b+cond_b bias folds into the weight row) -> PSUM [128, 2048];
    acts = tanh(a[:256]) * sigmoid(a[256:]) (ACT+DVE); res_skip 1x1 conv
    (2 matmuls/chunk/coutblk) -> PSUM; x/out updates via DVE
    scalar_tensor_tensor (fuses the rs bias add).
  - end conv (K=256 -> 8ch, channels reordered to [log_s, b]) + ACT bias,
    exp via ACT, coupling math via DVE, outputs DMA'd out per chunk.

Matmuls run as float32r (same bits as fp32, ~11-bit-mantissa PE path at full
bf16-rate throughput) — all matmul-feeding tensors are declared float32r
end-to-end so no cast copies exist anywhere. Measured ~0.57 ms/core on HW
with max relative error ~4e-4 vs the fp32 reference.
"""

import copy

import numpy as np

import concourse.bass as bass
import concourse.mybir as mybir
import concourse.tile as tile
from concourse import bass_utils
from concourse.vector_clock import ScopedClock

F32 = mybir.dt.float32
F32R = mybir.dt.float32r
AF = mybir.ActivationFunctionType
ALU = mybir.AluOpType

N_IN = 4
N_CTX = 80
L = 8
C = 256
DIL = [1, 2, 4, 8, 16, 32, 64, 128]
B, T = 8, 4096
TC = 512
NCHUNK = T // TC
PAD = 128
XW = PAD + T + PAD  # per cin-block padded width

MM_DT = mybir.dt.float32r  # matmul operand dtype tag


# ---------------------------------------------------------------------------
# Workarounds for the walrus build in this environment: it rejects any
# instruction carrying more than one sync-wait. Split extra waits onto
# single-wait carrier instructions placed just before the owner (same
# engine, so engine program order preserves semantics).
# ---------------------------------------------------------------------------


def _patched_drain_and_barrier(self, tick_clock, wait_clock):
    nc = self.nc
    tmp = nc.sync.nop(nofuse=True)
    wait_clock.add_sem_waits(tmp.ins, ScopedClock({None: tick_clock.global_clock}))
    si = tmp.ins.sync_info
    waits = list(si.on_wait) if si is not None else []
    if waits:
        si.on_wait = [waits[0]]
        for w in waits[1:]:
            n = nc.sync.nop(nofuse=True)
            n.ins.sync_info = mybir.SyncInfo(on_wait=[w], on_update=[])
    nc.sync.drain()
    nc.all_engine_barrier()
    popped = nc._tile_sem_poison_stack.pop()
    assert popped is self._sem_poison
    nc.clear_and_free_semaphores(list(self.sems.allocated().values()))
    nc.all_engine_barrier()


tile.TileContext._drain_and_barrier = _patched_drain_and_barrier


def split_multi_waits(nc):
    template = None
    for f in nc.m.functions:
        for bb in f.blocks:
            for inst in bb.instructions:
                if type(inst).__name__ == "InstEventSemaphore":
                    template = inst
                    break
            if template is not None:
                break
        if template is not None:
            break
    assert template is not None
    ctr = 0
    for f in nc.m.functions:
        for bb in f.blocks:
            insts = bb.instructions
            if not any(
                i.sync_info is not None and len(i.sync_info.on_wait) > 1
                for i in insts
            ):
                continue
            new = []
            for inst in insts:
                si = inst.sync_info
                if si is not None and len(si.on_wait) > 1:
                    waits = list(si.on_wait)
                    for w in waits[:-1]:
                        c = copy.copy(template)
                        c.name = f"waitsplit-{ctr}"
                        ctr += 1
                        c.engine = inst.engine
                        c.sync_info = mybir.SyncInfo(on_wait=[w], on_update=[])
                        new.append(c)
                    si.on_wait = [waits[-1]]
                new.append(inst)
            bb.instructions = new
    return ctr


# ---------------------------------------------------------------------------
# Program builder
# ---------------------------------------------------------------------------


def build_program():
    nc = bass.Bass("TRN2", target_bir_lowering=False, debug=False, num_devices=B)

    d_f0 = nc.dram_tensor("f0", [N_IN, T], F32R, kind="ExternalInput").ap()
    d_f1 = nc.dram_tensor("f1", [N_IN, T], F32, kind="ExternalInput").ap()
    d_ctx = nc.dram_tensor("ctxp", [N_CTX + 1, T], F32R, kind="ExternalInput").ap()
    d_win = nc.dram_tensor("w_in", [L, 128, 3072], F32R, kind="ExternalInput").ap()
    d_wrs = nc.dram_tensor("w_rs", [L, 128, 1024], F32R, kind="ExternalInput").ap()
    d_wcond = nc.dram_tensor("w_cond", [N_CTX + 1, 4096], F32R, kind="ExternalInput").ap()
    d_wstart = nc.dram_tensor("w_start", [N_IN, 256], F32R, kind="ExternalInput").ap()
    d_wend = nc.dram_tensor("w_end", [128, 16], F32R, kind="ExternalInput").ap()
    d_bend = nc.dram_tensor("b_end", [8, 1], F32, kind="ExternalInput").ap()
    d_rb = nc.dram_tensor("r_bias", [128, 4 * L], F32, kind="ExternalInput").ap()
    d_sb = nc.dram_tensor("s_bias", [128, 2], F32, kind="ExternalInput").ap()
    d_zeros = nc.dram_tensor("zeros", [128, 2 * PAD], F32R, kind="ExternalInput").ap()

    d_f1o = nc.dram_tensor("f1o", [N_IN, T], F32, kind="ExternalOutput").ap()
    d_logs = nc.dram_tensor("logs", [N_IN, T], F32, kind="ExternalOutput").ap()

    with tile.TileContext(nc) as tc:
        from contextlib import ExitStack

        ctx = ExitStack()
        with ctx:
            const = ctx.enter_context(tc.tile_pool(name="const", bufs=1))
            wpool = ctx.enter_context(tc.tile_pool(name="wpool", bufs=3))
            tspool = ctx.enter_context(tc.tile_pool(name="tspool", bufs=2))
            apool = ctx.enter_context(tc.tile_pool(name="apool", bufs=2))
            tailpool = ctx.enter_context(tc.tile_pool(name="tailpool", bufs=1))
            pspool = ctx.enter_context(tc.tile_pool(name="pspool", bufs=8, space="PSUM"))

            x = const.tile([128, 2 * XW], F32R, name="x")
            outacc = const.tile([128, 2 * T], F32R, name="outacc")
            ctxs = const.tile([N_CTX + 1, T], F32R, name="ctxs")
            f0s = const.tile([N_IN, T], F32R, name="f0s")
            conds = const.tile([N_CTX + 1, 4096], F32R, name="conds")
            starts = const.tile([N_IN, 256], F32R, name="starts")
            ends = const.tile([128, 16], F32R, name="ends")
            bendt = const.tile([8, 1], F32, name="bendt")
            rbt = const.tile([128, 4 * L], F32, name="rbt")
            sbt = const.tile([128, 2], F32, name="sbt")

            # PE warm-up: the HAM clock gate needs ~3.4us of sustained matmul
            # activity to lift the PE from 1.2 to 2.4 GHz. Burn the initial
            # DMA-wait window on dummy matmuls over a zeroed scratch tile so
            # the real matmuls start warm.
            scratch = const.tile([128, 64], F32, name="scratch")
            nc.vector.memset(scratch[:], 0.0)
            for w in range(10):
                ps_w = pspool.tile([64, 64], F32, tag="ps", name="ps_w")
                for _ in range(8):
                    nc.tensor.matmul(
                        ps_w[:], lhsT=scratch[:, 0:64], rhs=scratch[:], start=True, stop=True
                    )

            # critical-path loads on the SP queue (start conv inputs first)
            nc.sync.dma_start(starts[:], d_wstart[:])
            for j in range(NCHUNK):
                nc.sync.dma_start(
                    f0s[:, j * TC : (j + 1) * TC], d_f0[:, j * TC : (j + 1) * TC]
                )
            # layer-0 weights early on the SP queue so the first in-conv
            # matmuls don't wait behind anything else
            wint0 = wpool.tile([128, 3072], F32R, tag="win", name="wint0")
            nc.sync.dma_start(wint0[:], d_win[0])
            wrst0 = wpool.tile([128, 1024], F32R, tag="wrs", name="wrst0")
            nc.sync.dma_start(wrst0[:], d_wrs[0])
            nc.gpsimd.dma_start(sbt[:], d_sb[:])
            # bulk loads on the ACT HWDGE queue (start-conv copies were moved
            # to DVE so these descriptor writes don't block anything)
            nc.scalar.dma_start(rbt[:], d_rb[:])
            for j in range(NCHUNK):
                nc.scalar.dma_start(
                    ctxs[:, j * TC : (j + 1) * TC], d_ctx[:, j * TC : (j + 1) * TC]
                )
            for i in range(L):
                nc.scalar.dma_start(
                    conds[:, i * TC : (i + 1) * TC], d_wcond[:, i * TC : (i + 1) * TC]
                )
            nc.scalar.dma_start(ends[:], d_wend[:])
            nc.scalar.dma_start(bendt[:], d_bend[:])

            # zero the halo columns of x once; updates only touch the center
            # (DMA'd zeros — DVE memset can't write float32r)
            nc.gpsimd.dma_start(x[:, 0:PAD], d_zeros[:, 0:PAD])
            nc.gpsimd.dma_start(x[:, PAD + T : XW + PAD], d_zeros[:])
            nc.gpsimd.dma_start(x[:, XW + PAD + T : 2 * XW], d_zeros[:, 0:PAD])

            # ---- start conv: x = start_w @ f0 + start_b ----
            for j in range(NCHUNK):
                ps_s = [
                    pspool.tile([128, TC], F32, tag="ps", name=f"ps_s{c}")
                    for c in range(2)
                ]
                for c in range(2):
                    nc.tensor.matmul(
                        ps_s[c][:],
                        lhsT=starts[:, c * 128 : (c + 1) * 128],
                        rhs=f0s[:, j * TC : (j + 1) * TC],
                        start=True,
                        stop=True,
                    )
                for c in range(2):
                    nc.vector.tensor_scalar_add(
                        x[:, c * XW + PAD + j * TC : c * XW + PAD + (j + 1) * TC],
                        ps_s[c][:],
                        sbt[:, c : c + 1],
                    )

            # ---- WN layers ----
            # Software-pipelined: the gating/res-skip/update work for chunk j
            # is emitted AFTER chunk j+1's in-conv matmuls, so (a) the PE
            # stream never waits on the ACT->DVE gating chain, and (b) the
            # x update for chunk j lands after chunk j+1's k=0 tap has read
            # the pre-update tail (correctness of the dilated conv halo).
            def chunk_tail(st_):
                i, j, ps_a, wrst, last = (
                    st_["i"],
                    st_["j"],
                    st_["ps_a"],
                    st_["wrst"],
                    st_["last"],
                )
                tt = tspool.tile([128, 1024], F32, tag="tt", name="tt")
                st = tspool.tile([128, 1024], F32, tag="st", name="st")
                for c in range(2):
                    nc.scalar.activation(
                        tt[:, c * TC : (c + 1) * TC], ps_a[c][:], AF.Tanh
                    )
                    nc.scalar.activation(
                        st[:, c * TC : (c + 1) * TC], ps_a[2 + c][:], AF.Sigmoid
                    )
                actst = apool.tile([128, 1024], F32R, tag="acts", name="actst")
                nc.vector.tensor_mul(actst[:], tt[:], st[:])

                nco = 2 if last else 4
                ps_r = [
                    pspool.tile([128, TC], F32, tag="ps", name=f"ps_r{c2}")
                    for c2 in range(nco)
                ]
                for c2 in range(nco):
                    for p in range(2):
                        nc.tensor.matmul(
                            ps_r[c2][:],
                            lhsT=wrst[
                                :,
                                p * TC + c2 * 128 : p * TC + (c2 + 1) * 128,
                            ],
                            rhs=actst[:, p * TC : (p + 1) * TC],
                            start=(p == 0),
                            stop=(p == 1),
                        )
                if not last:
                    for c2 in range(2):
                        xs = x[
                            :, c2 * XW + PAD + j * TC : c2 * XW + PAD + (j + 1) * TC
                        ]
                        nc.vector.scalar_tensor_tensor(
                            xs,
                            ps_r[c2][:],
                            rbt[:, i * 4 + c2 : i * 4 + c2 + 1],
                            xs,
                            ALU.add,
                            ALU.add,
                        )
                    for c2 in range(2, 4):
                        os_ = outacc[
                            :, (c2 - 2) * T + j * TC : (c2 - 2) * T + (j + 1) * TC
                        ]
                        if i == 0:
                            nc.vector.tensor_scalar_add(
                                os_,
                                ps_r[c2][:],
                                rbt[:, i * 4 + c2 : i * 4 + c2 + 1],
                            )
                        else:
                            nc.vector.scalar_tensor_tensor(
                                os_,
                                ps_r[c2][:],
                                rbt[:, i * 4 + c2 : i * 4 + c2 + 1],
                                os_,
                                ALU.add,
                                ALU.add,
                            )
                else:
                    for c2 in range(2):
                        os_ = outacc[:, c2 * T + j * TC : c2 * T + (j + 1) * TC]
                        nc.vector.scalar_tensor_tensor(
                            os_,
                            ps_r[c2][:],
                            rbt[:, i * 4 + c2 : i * 4 + c2 + 1],
                            os_,
                            ALU.add,
                            ALU.add,
                        )

            # ---- end conv + coupling (emitted per-chunk, interleaved) ----
            def end_chunk(j):
                ps_e = pspool.tile([8, TC], F32, tag="ps", name="ps_e")
                for p in range(2):
                    nc.tensor.matmul(
                        ps_e[:],
                        lhsT=ends[:, p * 8 : (p + 1) * 8],
                        rhs=outacc[:, p * T + j * TC : p * T + (j + 1) * TC],
                        start=(p == 0),
                        stop=(p == 1),
                    )
                esb = tailpool.tile([8, TC], F32, tag="esb", name="esb")
                nc.scalar.activation(esb[:], ps_e[:], AF.Identity, bias=bendt[:])
                nc.sync.dma_start(d_logs[:, j * TC : (j + 1) * TC], esb[0:N_IN, :])
                expt = tailpool.tile([N_IN, TC], F32, tag="expt", name="expt")
                nc.scalar.activation(expt[:], esb[0:N_IN, :], AF.Exp)
                bsh = tailpool.tile([N_IN, TC], F32, tag="bsh", name="bsh")
                nc.sync.dma_start(bsh[:], esb[N_IN : 2 * N_IN, :])
                f1c = tailpool.tile([N_IN, TC], F32, tag="f1c", name="f1c")
                nc.sync.dma_start(f1c[:], d_f1[:, j * TC : (j + 1) * TC])
                f1oc = tailpool.tile([N_IN, TC], F32, tag="f1oc", name="f1oc")
                nc.vector.tensor_mul(f1oc[:], expt[:], f1c[:])
                nc.vector.tensor_add(f1oc[:], f1oc[:], bsh[:])
                nc.sync.dma_start(d_f1o[:, j * TC : (j + 1) * TC], f1oc[:])


            prev = None  # pipeline state carried across chunks AND layers
            for i in range(L):
                d = DIL[i]
                last = i == L - 1
                if i == 0:
                    wint, wrst = wint0, wrst0
                else:
                    wint = wpool.tile([128, 3072], F32R, tag="win", name="wint")
                    nc.sync.dma_start(wint[:], d_win[i])
                    wrst = wpool.tile([128, 1024], F32R, tag="wrs", name="wrst")
                    nc.sync.dma_start(wrst[:], d_wrs[i])

                for j in range(NCHUNK):
                    ps_a = [
                        pspool.tile([128, TC], F32, tag="ps", name=f"ps_a{c}")
                        for c in range(4)
                    ]
                    for c in range(4):
                        po = ps_a[c][:]
                        for k in range(3):
                            off = PAD + j * TC + (k - 1) * d
                            for p in range(2):
                                nc.tensor.matmul(
                                    po,
                                    lhsT=wint[
                                        :,
                                        (k * 2 + p) * TC
                                        + c * 128 : (k * 2 + p) * TC
                                        + (c + 1) * 128,
                                    ],
                                    rhs=x[:, p * XW + off : p * XW + off + TC],
                                    start=(k == 0 and p == 0),
                                    stop=False,
                                )
                        nc.tensor.matmul(
                            po,
                            lhsT=conds[:, i * TC + c * 128 : i * TC + (c + 1) * 128],
                            rhs=ctxs[:, j * TC : (j + 1) * TC],
                            start=False,
                            stop=True,
                        )
                    if prev is not None:
                        chunk_tail(prev)
                        if prev["last"]:
                            end_chunk(prev["j"])
                    prev = {"i": i, "j": j, "ps_a": ps_a, "wrst": wrst, "last": last}
            chunk_tail(prev)
            end_chunk(prev["j"])
            prev = None

    split_multi_waits(nc)
    return nc


# ---------------------------------------------------------------------------
# Host-side weight/layout prep
# ---------------------------------------------------------------------------


def prep_shared(inputs):
    f32 = np.float32
    in_w = np.ascontiguousarray(inputs["in_w"], f32)  # [8, 512, 256, 3]
    w_in = np.ascontiguousarray(
        in_w.reshape(L, 2 * C, 2, 128, 3).transpose(0, 3, 4, 2, 1).reshape(L, 128, 3072)
    )
    rs_w = np.ascontiguousarray(inputs["rs_w"], f32)[..., 0]  # [8, 512, 256]
    w_rs = np.ascontiguousarray(
        rs_w.reshape(L, 2 * C, 2, 128).transpose(0, 3, 2, 1).reshape(L, 128, 1024)
    )
    cond_w = np.ascontiguousarray(inputs["cond_w"], f32)[..., 0]  # [4096, 80]
    ab = inputs["in_b"].reshape(-1) + inputs["cond_b"]  # [4096]
    w_cond = np.concatenate([cond_w.T, ab[None, :]], axis=0).astype(f32)  # [81, 4096]
    w_start = np.ascontiguousarray(inputs["start_w"][..., 0].T, f32)  # [4, 256]
    perm = [N_IN + k for k in range(N_IN)] + list(range(N_IN))  # [log_s..., b...]
    ew = inputs["end_w"][..., 0][perm]  # [8, 256]
    w_end = np.ascontiguousarray(
        ew.T.reshape(2, 128, 8).transpose(1, 0, 2).reshape(128, 16), f32
    )
    b_end = np.ascontiguousarray(inputs["end_b"][perm][:, None], f32)  # [8, 1]
    r_bias = np.ascontiguousarray(
        inputs["rs_b"].reshape(L, 4, 128).transpose(2, 0, 1).reshape(128, 4 * L), f32
    )
    s_bias = np.ascontiguousarray(inputs["start_b"].reshape(2, 128).T, f32)  # [128, 2]
    return {
        "w_in": w_in,
        "w_rs": w_rs,
        "w_cond": w_cond,
        "w_start": w_start,
        "w_end": w_end,
        "b_end": b_end,
        "r_bias": r_bias,
        "s_bias": s_bias,
    }


def _ensure_ntff_hook():
    """Register the axon NTFF profiling hook if the image's antenv lacks it."""
    import sys
    import types

    try:
        import antenv.axon_hooks  # noqa: F401

        return
    except ImportError:
        pass
    mod = types.ModuleType("antenv.axon_hooks")
    holder = [None]
    mod.set_axon_ntff_profile_hook = lambda h: holder.__setitem__(0, h)
    mod.get_axon_ntff_profile_hook = lambda: holder[0]
    sys.modules["antenv.axon_hooks"] = mod
    try:
        from trn_agent_boot.trn_boot import _ntff_profile_via_ctypes

        mod.set_axon_ntff_profile_hook(
            _ntff_profile_via_ctypes("/opt/axon/libaxon_pjrt.so")
        )
    except Exception:
        pass


_NC = None


def _get_program():
    global _NC
    if _NC is None:
        _NC = build_program()
    return _NC


LAST_RESULTS = None


def kernel(forecast, context, start_w, start_b, cond_w, cond_b,
           in_w, in_b, rs_w, rs_b, end_w, end_b, _trace=False):
    global LAST_RESULTS
    inputs = dict(
        forecast=np.asarray(forecast), context=np.asarray(context),
        start_w=np.asarray(start_w), start_b=np.asarray(start_b),
        cond_w=np.asarray(cond_w), cond_b=np.asarray(cond_b),
        in_w=np.asarray(in_w), in_b=np.asarray(in_b),
        rs_w=np.asarray(rs_w), rs_b=np.asarray(rs_b),
        end_w=np.asarray(end_w), end_b=np.asarray(end_b),
    )
    shared = prep_shared(inputs)
    shared["zeros"] = np.zeros((128, 2 * PAD), np.float32)
    forecast = np.ascontiguousarray(inputs["forecast"], np.float32)
    context = np.ascontiguousarray(inputs["context"], np.float32)
    ones_row = np.ones((1, T), np.float32)

    in_maps = []
    for b in range(B):
        m = dict(shared)
        m["f0"] = np.ascontiguousarray(forecast[b, :N_IN])
        m["f1"] = np.ascontiguousarray(forecast[b, N_IN:])
        m["ctxp"] = np.ascontiguousarray(
            np.concatenate([context[b], ones_row], axis=0)
        )
        in_maps.append(m)

    if _trace:
        _ensure_ntff_hook()
    nc = _get_program()
    try:
        res = bass_utils.run_bass_kernel_spmd(
            nc, in_maps, core_ids=list(range(B)), trace=_trace
        )
    except Exception:
        # transient device errors (e.g. NRT_EXEC_UNIT_UNRECOVERABLE after a
        # prior crashed run) usually clear on retry with a fresh NRT session
        import time

        time.sleep(2.0)
        res = bass_utils.run_bass_kernel_spmd(
            nc, in_maps, core_ids=list(range(B)), trace=_trace
        )
    LAST_RESULTS = res

    f1o = np.stack([res.results[b]["f1o"] for b in range(B)])  # [8, 4, T]
    logs = np.stack([res.results[b]["logs"] for b in range(B)])  # [8, 4, T]
    out_full = np.concatenate([forecast[:, :N_IN], f1o], axis=1)  # [8, 8, T]
    return out_full, logs


# revision 28
# speedup vs baseline: 1.0057x; 1.0020x over previous
"""Trainium2 Bass kernel for nn_AffineCoupling (WaveGlow-style WN coupling).

Sharding: data-parallel over batch — B=8 samples, one per NeuronCore. All
convs are per-sample so no cross-core communication is needed.

Per-core plan (T=4096, chunked into 8 x 512 columns):
  - x (residual, 256ch) lives in SBUF as [128, 2*(128+4096+128)] fp32 with
    zero pads so dilated-conv taps are plain shifted column reads.
  - per layer: a = in_conv(x) (K=3 dilated, 6 matmuls/chunk/coutblk)
    + cond matmul (K=81: context padded with a ones-row so the combined
    in_b+cond_b bias folds into the weight row) -> PSUM [128, 2048];
    acts = tanh(a[:256]) * sigmoid(a[256:]) (ACT+DVE); res_skip 1x1 conv
    (2 matmuls/chunk/coutblk) -> PSUM; x/out updates via DVE
    scalar_tensor_tensor (fuses the rs bias add).
  - end conv (K=256 -> 8ch, channels reordered to [log_s, b]) + ACT bias,
    exp via ACT, coupling math via DVE, outputs DMA'd out per chunk.

Matmuls run as float32r (same bits as fp32, ~11-bit-mantissa PE path at full
bf16-rate throughput) — all matmul-feeding tensors are declared float32r
end-to-end so no cast copies exist anywhere. Measured ~0.57 ms/core on HW
with max relative error ~4e-4 vs the fp32 reference.
"""

import copy

import numpy as np

import concourse.bass as bass
import concourse.mybir as mybir
import concourse.tile as tile
from concourse import bass_utils
from concourse.vector_clock import ScopedClock

F32 = mybir.dt.float32
F32R = mybir.dt.float32r
AF = mybir.ActivationFunctionType
ALU = mybir.AluOpType

N_IN = 4
N_CTX = 80
L = 8
C = 256
DIL = [1, 2, 4, 8, 16, 32, 64, 128]
B, T = 8, 4096
TC = 512
NCHUNK = T // TC
PAD = 128
XW = PAD + T + PAD  # per cin-block padded width

MM_DT = mybir.dt.float32r  # matmul operand dtype tag


# ---------------------------------------------------------------------------
# Workarounds for the walrus build in this environment: it rejects any
# instruction carrying more than one sync-wait. Split extra waits onto
# single-wait carrier instructions placed just before the owner (same
# engine, so engine program order preserves semantics).
# ---------------------------------------------------------------------------


def _patched_drain_and_barrier(self, tick_clock, wait_clock):
    nc = self.nc
    tmp = nc.sync.nop(nofuse=True)
    wait_clock.add_sem_waits(tmp.ins, ScopedClock({None: tick_clock.global_clock}))
    si = tmp.ins.sync_info
    waits = list(si.on_wait) if si is not None else []
    if waits:
        si.on_wait = [waits[0]]
        for w in waits[1:]:
            n = nc.sync.nop(nofuse=True)
            n.ins.sync_info = mybir.SyncInfo(on_wait=[w], on_update=[])
    nc.sync.drain()
    nc.all_engine_barrier()
    popped = nc._tile_sem_poison_stack.pop()
    assert popped is self._sem_poison
    nc.clear_and_free_semaphores(list(self.sems.allocated().values()))
    nc.all_engine_barrier()


tile.TileContext._drain_and_barrier = _patched_drain_and_barrier


def split_multi_waits(nc):
    template = None
    for f in nc.m.functions:
        for bb in f.blocks:
            for inst in bb.instructions:
                if type(inst).__name__ == "InstEventSemaphore":
                    template = inst
                    break
            if template is not None:
                break
        if template is not None:
            break
    assert template is not None
    ctr = 0
    for f in nc.m.functions:
        for bb in f.blocks:
            insts = bb.instructions
            if not any(
                i.sync_info is not None and len(i.sync_info.on_wait) > 1
                for i in insts
            ):
                continue
            new = []
            for inst in insts:
                si = inst.sync_info
                if si is not None and len(si.on_wait) > 1:
                    waits = list(si.on_wait)
                    for w in waits[:-1]:
                        c = copy.copy(template)
                        c.name = f"waitsplit-{ctr}"
                        ctr += 1
                        c.engine = inst.engine
                        c.sync_info = mybir.SyncInfo(on_wait=[w], on_update=[])
                        new.append(c)
                    si.on_wait = [waits[-1]]
                new.append(inst)
            bb.instructions = new
    return ctr


# ---------------------------------------------------------------------------
# Program builder
# ---------------------------------------------------------------------------


def build_program():
    nc = bass.Bass("TRN2", target_bir_lowering=False, debug=False, num_devices=B)

    d_f0 = nc.dram_tensor("f0", [N_IN, T], F32R, kind="ExternalInput").ap()
    d_f1 = nc.dram_tensor("f1", [N_IN, T], F32, kind="ExternalInput").ap()
    d_ctx = nc.dram_tensor("ctxp", [N_CTX + 1, T], F32R, kind="ExternalInput").ap()
    d_win = nc.dram_tensor("w_in", [L, 128, 3072], F32R, kind="ExternalInput").ap()
    d_wrs = nc.dram_tensor("w_rs", [L, 128, 1024], F32R, kind="ExternalInput").ap()
    d_wcond = nc.dram_tensor("w_cond", [N_CTX + 1, 4096], F32R, kind="ExternalInput").ap()
    d_wstart = nc.dram_tensor("w_start", [N_IN, 256], F32R, kind="ExternalInput").ap()
    d_wend = nc.dram_tensor("w_end", [128, 16], F32R, kind="ExternalInput").ap()
    d_bend = nc.dram_tensor("b_end", [8, 1], F32, kind="ExternalInput").ap()
    d_rb = nc.dram_tensor("r_bias", [128, 4 * L], F32, kind="ExternalInput").ap()
    d_sb = nc.dram_tensor("s_bias", [128, 2], F32, kind="ExternalInput").ap()
    d_zeros = nc.dram_tensor("zeros", [128, 2 * PAD], F32R, kind="ExternalInput").ap()

    d_f1o = nc.dram_tensor("f1o", [N_IN, T], F32, kind="ExternalOutput").ap()
    d_logs = nc.dram_tensor("logs", [N_IN, T], F32, kind="ExternalOutput").ap()

    with tile.TileContext(nc) as tc:
        from contextlib import ExitStack

        ctx = ExitStack()
        with ctx:
            const = ctx.enter_context(tc.tile_pool(name="const", bufs=1))
            wpool = ctx.enter_context(tc.tile_pool(name="wpool", bufs=3))
            tspool = ctx.enter_context(tc.tile_pool(name="tspool", bufs=2))
            apool = ctx.enter_context(tc.tile_pool(name="apool", bufs=2))
            tailpool = ctx.enter_context(tc.tile_pool(name="tailpool", bufs=1))
            pspool = ctx.enter_context(tc.tile_pool(name="pspool", bufs=8, space="PSUM"))

            x = const.tile([128, 2 * XW], F32R, name="x")
            outacc = const.tile([128, 2 * T], F32R, name="outacc")
            ctxs = const.tile([N_CTX + 1, T], F32R, name="ctxs")
            f0s = const.tile([N_IN, T], F32R, name="f0s")
            conds = const.tile([N_CTX + 1, 4096], F32R, name="conds")
            starts = const.tile([N_IN, 256], F32R, name="starts")
            ends = const.tile([128, 16], F32R, name="ends")
            bendt = const.tile([8, 1], F32, name="bendt")
            rbt = const.tile([128, 4 * L], F32, name="rbt")
            sbt = const.tile([128, 2], F32, name="sbt")

            # PE warm-up: the HAM clock gate needs ~3.4us of sustained matmul
            # activity to lift the PE from 1.2 to 2.4 GHz. Burn the initial
            # DMA-wait window on dummy matmuls over a zeroed scratch tile so
            # the real matmuls start warm.
            scratch = const.tile([128, 64], F32, name="scratch")
            nc.vector.memset(scratch[:], 0.0)
            for w in range(3):
                ps_w = pspool.tile([64, 64], F32, tag="ps", name="ps_w")
                for _ in range(8):
                    nc.tensor.matmul(
                        ps_w[:], lhsT=scratch[:, 0:64], rhs=scratch[:], start=True, stop=True
                    )

            # critical-path loads on the SP queue (start conv inputs first)
            nc.sync.dma_start(starts[:], d_wstart[:])
            for j in range(NCHUNK):
                nc.sync.dma_start(
                    f0s[:, j * TC : (j + 1) * TC], d_f0[:, j * TC : (j + 1) * TC]
                )
            # layer-0 weights early on the SP queue so the first in-conv
            # matmuls don't wait behind anything else
            wint0 = wpool.tile([128, 3072], F32R, tag="win", name="wint0")
            nc.sync.dma_start(wint0[:], d_win[0])
            wrst0 = wpool.tile([128, 1024], F32R, tag="wrs", name="wrst0")
            nc.sync.dma_start(wrst0[:], d_wrs[0])
            nc.gpsimd.dma_start(sbt[:], d_sb[:])
            # bulk loads on the ACT HWDGE queue (start-conv copies were moved
            # to DVE so these descriptor writes don't block anything)
            nc.scalar.dma_start(rbt[:], d_rb[:])
            for j in range(NCHUNK):
                nc.scalar.dma_start(
                    ctxs[:, j * TC : (j + 1) * TC], d_ctx[:, j * TC : (j + 1) * TC]
                )
            for i in range(L):
                nc.scalar.dma_start(
                    conds[:, i * TC : (i + 1) * TC], d_wcond[:, i * TC : (i + 1) * TC]
                )
            nc.scalar.dma_start(ends[:], d_wend[:])
            nc.scalar.dma_start(bendt[:], d_bend[:])

            # zero the halo columns of x once; updates only touch the center
            # (DMA'd zeros — DVE memset can't write float32r)
            nc.gpsimd.dma_start(x[:, 0:PAD], d_zeros[:, 0:PAD])
            nc.gpsimd.dma_start(x[:, PAD + T : XW + PAD], d_zeros[:])
            nc.gpsimd.dma_start(x[:, XW + PAD + T : 2 * XW], d_zeros[:, 0:PAD])

            # ---- start conv: x = start_w @ f0 + start_b ----
            for j in range(NCHUNK):
                ps_s = [
                    pspool.tile([128, TC], F32, tag="ps", name=f"ps_s{c}")
                    for c in range(2)
                ]
                for c in range(2):
                    nc.tensor.matmul(
                        ps_s[c][:],
                        lhsT=starts[:, c * 128 : (c + 1) * 128],
                        rhs=f0s[:, j * TC : (j + 1) * TC],
                        start=True,
                        stop=True,
                    )
                for c in range(2):
                    nc.vector.tensor_scalar_add(
                        x[:, c * XW + PAD + j * TC : c * XW + PAD + (j + 1) * TC],
                        ps_s[c][:],
                        sbt[:, c : c + 1],
                    )

            # ---- WN layers ----
            # Software-pipelined: the gating/res-skip/update work for chunk j
            # is emitted AFTER chunk j+1's in-conv matmuls, so (a) the PE
            # stream never waits on the ACT->DVE gating chain, and (b) the
            # x update for chunk j lands after chunk j+1's k=0 tap has read
            # the pre-update tail (correctness of the dilated conv halo).
            def chunk_tail(st_):
                i, j, ps_a, wrst, last = (
                    st_["i"],
                    st_["j"],
                    st_["ps_a"],
                    st_["wrst"],
                    st_["last"],
                )
                tt = tspool.tile([128, 1024], F32, tag="tt", name="tt")
                st = tspool.tile([128, 1024], F32, tag="st", name="st")
                for c in range(2):
                    nc.scalar.activation(
                        tt[:, c * TC : (c + 1) * TC], ps_a[c][:], AF.Tanh
                    )
                    nc.scalar.activation(
                        st[:, c * TC : (c + 1) * TC], ps_a[2 + c][:], AF.Sigmoid
                    )
                actst = apool.tile([128, 1024], F32R, tag="acts", name="actst")
                nc.vector.tensor_mul(actst[:], tt[:], st[:])

                nco = 2 if last else 4
                ps_r = [
                    pspool.tile([128, TC], F32, tag="ps", name=f"ps_r{c2}")
                    for c2 in range(nco)
                ]
                for c2 in range(nco):
                    for p in range(2):
                        nc.tensor.matmul(
                            ps_r[c2][:],
                            lhsT=wrst[
                                :,
                                p * TC + c2 * 128 : p * TC + (c2 + 1) * 128,
                            ],
                            rhs=actst[:, p * TC : (p + 1) * TC],
                            start=(p == 0),
                            stop=(p == 1),
                        )
                if not last:
                    for c2 in range(2):
                        xs = x[
                            :, c2 * XW + PAD + j * TC : c2 * XW + PAD + (j + 1) * TC
                        ]
                        nc.vector.scalar_tensor_tensor(
                            xs,
                            ps_r[c2][:],
                            rbt[:, i * 4 + c2 : i * 4 + c2 + 1],
                            xs,
                            ALU.add,
                            ALU.add,
                        )
                    for c2 in range(2, 4):
                        os_ = outacc[
                            :, (c2 - 2) * T + j * TC : (c2 - 2) * T + (j + 1) * TC
                        ]
                        if i == 0:
                            nc.vector.tensor_scalar_add(
                                os_,
                                ps_r[c2][:],
                                rbt[:, i * 4 + c2 : i * 4 + c2 + 1],
                            )
                        else:
                            nc.vector.scalar_tensor_tensor(
                                os_,
                                ps_r[c2][:],
                                rbt[:, i * 4 + c2 : i * 4 + c2 + 1],
                                os_,
                                ALU.add,
                                ALU.add,
                            )
                else:
                    for c2 in range(2):
                        os_ = outacc[:, c2 * T + j * TC : c2 * T + (j + 1) * TC]
                        nc.vector.scalar_tensor_tensor(
                            os_,
                            ps_r[c2][:],
                            rbt[:, i * 4 + c2 : i * 4 + c2 + 1],
                            os_,
                            ALU.add,
                            ALU.add,
                        )

            # ---- end conv + coupling (emitted per-chunk, interleaved) ----
            def end_chunk(j):
                ps_e = pspool.tile([8, TC], F32, tag="ps", name="ps_e")
                for p in range(2):
                    nc.tensor.matmul(
                        ps_e[:],
                        lhsT=ends[:, p * 8 : (p + 1) * 8],
                        rhs=outacc[:, p * T + j * TC : p * T + (j + 1) * TC],
                        start=(p == 0),
                        stop=(p == 1),
                    )
                esb = tailpool.tile([8, TC], F32, tag="esb", name="esb")
                nc.scalar.activation(esb[:], ps_e[:], AF.Identity, bias=bendt[:])
                nc.sync.dma_start(d_logs[:, j * TC : (j + 1) * TC], esb[0:N_IN, :])
                expt = tailpool.tile([N_IN, TC], F32, tag="expt", name="expt")
                nc.scalar.activation(expt[:], esb[0:N_IN, :], AF.Exp)
                bsh = tailpool.tile([N_IN, TC], F32, tag="bsh", name="bsh")
                nc.sync.dma_start(bsh[:], esb[N_IN : 2 * N_IN, :])
                f1c = tailpool.tile([N_IN, TC], F32, tag="f1c", name="f1c")
                nc.sync.dma_start(f1c[:], d_f1[:, j * TC : (j + 1) * TC])
                f1oc = tailpool.tile([N_IN, TC], F32, tag="f1oc", name="f1oc")
                nc.vector.tensor_mul(f1oc[:], expt[:], f1c[:])
                nc.vector.tensor_add(f1oc[:], f1oc[:], bsh[:])
                nc.sync.dma_start(d_f1o[:, j * TC : (j + 1) * TC], f1oc[:])


            prev = None  # pipeline state carried across chunks AND layers
            for i in range(L):
                d = DIL[i]
                last = i == L - 1
                if i == 0:
                    wint, wrst = wint0, wrst0
                else:
                    wint = wpool.tile([128, 3072], F32R, tag="win", name="wint")
                    nc.sync.dma_start(wint[:], d_win[i])
                    wrst = wpool.tile([128, 1024], F32R, tag="wrs", name="wrst")
                    nc.sync.dma_start(wrst[:], d_wrs[i])

                for j in range(NCHUNK):
                    ps_a = [
                        pspool.tile([128, TC], F32, tag="ps", name=f"ps_a{c}")
                        for c in range(4)
                    ]
                    for c in range(4):
                        po = ps_a[c][:]
                        for k in range(3):
                            off = PAD + j * TC + (k - 1) * d
                            for p in range(2):
                                nc.tensor.matmul(
                                    po,
                                    lhsT=wint[
                                        :,
                                        (k * 2 + p) * TC
                                        + c * 128 : (k * 2 + p) * TC
                                        + (c + 1) * 128,
                                    ],
                                    rhs=x[:, p * XW + off : p * XW + off + TC],
                                    start=(k == 0 and p == 0),
                                    stop=False,
                                )
                        nc.tensor.matmul(
                            po,
                            lhsT=conds[:, i * TC + c * 128 : i * TC + (c + 1) * 128],
                            rhs=ctxs[:, j * TC : (j + 1) * TC],
                            start=False,
                            stop=True,
                        )
                    if prev is not None:
                        chunk_tail(prev)
                        if prev["last"]:
                            end_chunk(prev["j"])
                    prev = {"i": i, "j": j, "ps_a": ps_a, "wrst": wrst, "last": last}
            chunk_tail(prev)
            end_chunk(prev["j"])
            prev = None

    split_multi_waits(nc)
    return nc


# ---------------------------------------------------------------------------
# Host-side weight/layout prep
# ---------------------------------------------------------------------------


def prep_shared(inputs):
    f32 = np.float32
    in_w = np.ascontiguousarray(inputs["in_w"], f32)  # [8, 512, 256, 3]
    w_in = np.ascontiguousarray(
        in_w.reshape(L, 2 * C, 2, 128, 3).transpose(0, 3, 4, 2, 1).reshape(L, 128, 3072)
    )
    rs_w = np.ascontiguousarray(inputs["rs_w"], f32)[..., 0]  # [8, 512, 256]
    w_rs = np.ascontiguousarray(
        rs_w.reshape(L, 2 * C, 2, 128).transpose(0, 3, 2, 1).reshape(L, 128, 1024)
    )
    cond_w = np.ascontiguousarray(inputs["cond_w"], f32)[..., 0]  # [4096, 80]
    ab = inputs["in_b"].reshape(-1) + inputs["cond_b"]  # [4096]
    w_cond = np.concatenate([cond_w.T, ab[None, :]], axis=0).astype(f32)  # [81, 4096]
    w_start = np.ascontiguousarray(inputs["start_w"][..., 0].T, f32)  # [4, 256]
    perm = [N_IN + k for k in range(N_IN)] + list(range(N_IN))  # [log_s..., b...]
    ew = inputs["end_w"][..., 0][perm]  # [8, 256]
    w_end = np.ascontiguousarray(
        ew.T.reshape(2, 128, 8).transpose(1, 0, 2).reshape(128, 16), f32
    )
    b_end = np.ascontiguousarray(inputs["end_b"][perm][:, None], f32)  # [8, 1]
    r_bias = np.ascontiguousarray(
        inputs["rs_b"].reshape(L, 4, 128).transpose(2, 0, 1).reshape(128, 4 * L), f32
    )
    s_bias = np.ascontiguousarray(inputs["start_b"].reshape(2, 128).T, f32)  # [128, 2]
    return {
        "w_in": w_in,
        "w_rs": w_rs,
        "w_cond": w_cond,
        "w_start": w_start,
        "w_end": w_end,
        "b_end": b_end,
        "r_bias": r_bias,
        "s_bias": s_bias,
    }


def _ensure_ntff_hook():
    """Register the axon NTFF profiling hook if the image's antenv lacks it."""
    import sys
    import types

    try:
        import antenv.axon_hooks  # noqa: F401

        return
    except ImportError:
        pass
    mod = types.ModuleType("antenv.axon_hooks")
    holder = [None]
    mod.set_axon_ntff_profile_hook = lambda h: holder.__setitem__(0, h)
    mod.get_axon_ntff_profile_hook = lambda: holder[0]
    sys.modules["antenv.axon_hooks"] = mod
    try:
        from trn_agent_boot.trn_boot import _ntff_profile_via_ctypes

        mod.set_axon_ntff_profile_hook(
            _ntff_profile_via_ctypes("/opt/axon/libaxon_pjrt.so")
        )
    except Exception:
        pass


_NC = None


def _get_program():
    global _NC
    if _NC is None:
        _NC = build_program()
    return _NC


LAST_RESULTS = None


def kernel(forecast, context, start_w, start_b, cond_w, cond_b,
           in_w, in_b, rs_w, rs_b, end_w, end_b, _trace=False):
    global LAST_RESULTS
    inputs = dict(
        forecast=np.asarray(forecast), context=np.asarray(context),
        start_w=np.asarray(start_w), start_b=np.asarray(start_b),
        cond_w=np.asarray(cond_w), cond_b=np.asarray(cond_b),
        in_w=np.asarray(in_w), in_b=np.asarray(in_b),
        rs_w=np.asarray(rs_w), rs_b=np.asarray(rs_b),
        end_w=np.asarray(end_w), end_b=np.asarray(end_b),
    )
    shared = prep_shared(inputs)
    shared["zeros"] = np.zeros((128, 2 * PAD), np.float32)
    forecast = np.ascontiguousarray(inputs["forecast"], np.float32)
    context = np.ascontiguousarray(inputs["context"], np.float32)
    ones_row = np.ones((1, T), np.float32)

    in_maps = []
    for b in range(B):
        m = dict(shared)
        m["f0"] = np.ascontiguousarray(forecast[b, :N_IN])
        m["f1"] = np.ascontiguousarray(forecast[b, N_IN:])
        m["ctxp"] = np.ascontiguousarray(
            np.concatenate([context[b], ones_row], axis=0)
        )
        in_maps.append(m)

    if _trace:
        _ensure_ntff_hook()
    nc = _get_program()
    try:
        res = bass_utils.run_bass_kernel_spmd(
            nc, in_maps, core_ids=list(range(B)), trace=_trace
        )
    except Exception:
        # transient device errors (e.g. NRT_EXEC_UNIT_UNRECOVERABLE after a
        # prior crashed run) usually clear on retry with a fresh NRT session
        import time

        time.sleep(2.0)
        res = bass_utils.run_bass_kernel_spmd(
            nc, in_maps, core_ids=list(range(B)), trace=_trace
        )
    LAST_RESULTS = res

    f1o = np.stack([res.results[b]["f1o"] for b in range(B)])  # [8, 4, T]
    logs = np.stack([res.results[b]["logs"] for b in range(B)])  # [8, 4, T]
    out_full = np.concatenate([forecast[:, :N_IN], f1o], axis=1)  # [8, 8, T]
    return out_full, logs


# revision 29
# speedup vs baseline: 1.0102x; 1.0045x over previous
"""Trainium2 Bass kernel for nn_AffineCoupling (WaveGlow-style WN coupling).

Sharding: data-parallel over batch — B=8 samples, one per NeuronCore. All
convs are per-sample so no cross-core communication is needed.

Per-core plan (T=4096, chunked into 8 x 512 columns):
  - x (residual, 256ch) lives in SBUF as [128, 2*(128+4096+128)] fp32 with
    zero pads so dilated-conv taps are plain shifted column reads.
  - per layer: a = in_conv(x) (K=3 dilated, 6 matmuls/chunk/coutblk)
    + cond matmul (K=81: context padded with a ones-row so the combined
    in_b+cond_b bias folds into the weight row) -> PSUM [128, 2048];
    acts = tanh(a[:256]) * sigmoid(a[256:]) (ACT+DVE); res_skip 1x1 conv
    (2 matmuls/chunk/coutblk) -> PSUM; x/out updates via DVE
    scalar_tensor_tensor (fuses the rs bias add).
  - end conv (K=256 -> 8ch, channels reordered to [log_s, b]) + ACT bias,
    exp via ACT, coupling math via DVE, outputs DMA'd out per chunk.

Matmuls run as float32r (same bits as fp32, ~11-bit-mantissa PE path at full
bf16-rate throughput) — all matmul-feeding tensors are declared float32r
end-to-end so no cast copies exist anywhere. Measured ~0.57 ms/core on HW
with max relative error ~4e-4 vs the fp32 reference.
"""

import copy

import numpy as np

import concourse.bass as bass
import concourse.mybir as mybir
import concourse.tile as tile
from concourse import bass_utils
from concourse.vector_clock import ScopedClock

F32 = mybir.dt.float32
F32R = mybir.dt.float32r
AF = mybir.ActivationFunctionType
ALU = mybir.AluOpType

N_IN = 4
N_CTX = 80
L = 8
C = 256
DIL = [1, 2, 4, 8, 16, 32, 64, 128]
B, T = 8, 4096
TC = 512
NCHUNK = T // TC
PAD = 128
XW = PAD + T + PAD  # per cin-block padded width

MM_DT = mybir.dt.float32r  # matmul operand dtype tag


# ---------------------------------------------------------------------------
# Workarounds for the walrus build in this environment: it rejects any
# instruction carrying more than one sync-wait. Split extra waits onto
# single-wait carrier instructions placed just before the owner (same
# engine, so engine program order preserves semantics).
# ---------------------------------------------------------------------------


def _patched_drain_and_barrier(self, tick_clock, wait_clock):
    nc = self.nc
    tmp = nc.sync.nop(nofuse=True)
    wait_clock.add_sem_waits(tmp.ins, ScopedClock({None: tick_clock.global_clock}))
    si = tmp.ins.sync_info
    waits = list(si.on_wait) if si is not None else []
    if waits:
        si.on_wait = [waits[0]]
        for w in waits[1:]:
            n = nc.sync.nop(nofuse=True)
            n.ins.sync_info = mybir.SyncInfo(on_wait=[w], on_update=[])
    nc.sync.drain()
    nc.all_engine_barrier()
    popped = nc._tile_sem_poison_stack.pop()
    assert popped is self._sem_poison
    nc.clear_and_free_semaphores(list(self.sems.allocated().values()))
    nc.all_engine_barrier()


tile.TileContext._drain_and_barrier = _patched_drain_and_barrier


def split_multi_waits(nc):
    template = None
    for f in nc.m.functions:
        for bb in f.blocks:
            for inst in bb.instructions:
                if type(inst).__name__ == "InstEventSemaphore":
                    template = inst
                    break
            if template is not None:
                break
        if template is not None:
            break
    assert template is not None
    ctr = 0
    for f in nc.m.functions:
        for bb in f.blocks:
            insts = bb.instructions
            if not any(
                i.sync_info is not None and len(i.sync_info.on_wait) > 1
                for i in insts
            ):
                continue
            new = []
            for inst in insts:
                si = inst.sync_info
                if si is not None and len(si.on_wait) > 1:
                    waits = list(si.on_wait)
                    for w in waits[:-1]:
                        c = copy.copy(template)
                        c.name = f"waitsplit-{ctr}"
                        ctr += 1
                        c.engine = inst.engine
                        c.sync_info = mybir.SyncInfo(on_wait=[w], on_update=[])
                        new.append(c)
                    si.on_wait = [waits[-1]]
                new.append(inst)
            bb.instructions = new
    return ctr


# ---------------------------------------------------------------------------
# Program builder
# ---------------------------------------------------------------------------


def build_program():
    nc = bass.Bass("TRN2", target_bir_lowering=False, debug=False, num_devices=B)

    d_f0 = nc.dram_tensor("f0", [N_IN, T], F32R, kind="ExternalInput").ap()
    d_f1 = nc.dram_tensor("f1", [N_IN, T], F32, kind="ExternalInput").ap()
    d_ctx = nc.dram_tensor("ctxp", [N_CTX + 1, T], F32R, kind="ExternalInput").ap()
    d_win = nc.dram_tensor("w_in", [L, 128, 3072], F32R, kind="ExternalInput").ap()
    d_wrs = nc.dram_tensor("w_rs", [L, 128, 1024], F32R, kind="ExternalInput").ap()
    d_wcond = nc.dram_tensor("w_cond", [N_CTX + 1, 4096], F32R, kind="ExternalInput").ap()
    d_wstart = nc.dram_tensor("w_start", [N_IN, 256], F32R, kind="ExternalInput").ap()
    d_wend = nc.dram_tensor("w_end", [128, 16], F32R, kind="ExternalInput").ap()
    d_bend = nc.dram_tensor("b_end", [8, 1], F32, kind="ExternalInput").ap()
    d_rb = nc.dram_tensor("r_bias", [128, 4 * L], F32, kind="ExternalInput").ap()
    d_sb = nc.dram_tensor("s_bias", [128, 2], F32, kind="ExternalInput").ap()
    d_zeros = nc.dram_tensor("zeros", [128, 2 * PAD], F32R, kind="ExternalInput").ap()

    d_f1o = nc.dram_tensor("f1o", [N_IN, T], F32, kind="ExternalOutput").ap()
    d_logs = nc.dram_tensor("logs", [N_IN, T], F32, kind="ExternalOutput").ap()

    with tile.TileContext(nc) as tc:
        from contextlib import ExitStack

        ctx = ExitStack()
        with ctx:
            const = ctx.enter_context(tc.tile_pool(name="const", bufs=1))
            wpool = ctx.enter_context(tc.tile_pool(name="wpool", bufs=3))
            tspool = ctx.enter_context(tc.tile_pool(name="tspool", bufs=2))
            apool = ctx.enter_context(tc.tile_pool(name="apool", bufs=2))
            tailpool = ctx.enter_context(tc.tile_pool(name="tailpool", bufs=1))
            pspool = ctx.enter_context(tc.tile_pool(name="pspool", bufs=8, space="PSUM"))

            x = const.tile([128, 2 * XW], F32R, name="x")
            outacc = const.tile([128, 2 * T], F32R, name="outacc")
            ctxs = const.tile([N_CTX + 1, T], F32R, name="ctxs")
            f0s = const.tile([N_IN, T], F32R, name="f0s")
            conds = const.tile([N_CTX + 1, 4096], F32R, name="conds")
            starts = const.tile([N_IN, 256], F32R, name="starts")
            ends = const.tile([128, 16], F32R, name="ends")
            bendt = const.tile([8, 1], F32, name="bendt")
            rbt = const.tile([128, 4 * L], F32, name="rbt")
            sbt = const.tile([128, 2], F32, name="sbt")

            # PE warm-up: the HAM clock gate needs ~3.4us of sustained matmul
            # activity to lift the PE from 1.2 to 2.4 GHz. Burn the initial
            # DMA-wait window on dummy matmuls over a zeroed scratch tile so
            # the real matmuls start warm.
            scratch = const.tile([128, 64], F32, name="scratch")
            nc.vector.memset(scratch[:], 0.0)
            for w in range(3):
                ps_w = pspool.tile([64, 64], F32, tag="ps", name="ps_w")
                for _ in range(8):
                    nc.tensor.matmul(
                        ps_w[:], lhsT=scratch[:, 0:64], rhs=scratch[:], start=True, stop=True
                    )

            # critical-path loads on the SP queue (start conv inputs first)
            nc.sync.dma_start(starts[:], d_wstart[:])
            nc.sync.dma_start(f0s[:], d_f0[:])
            # layer-0 weights early on the SP queue so the first in-conv
            # matmuls don't wait behind anything else
            wint0 = wpool.tile([128, 3072], F32R, tag="win", name="wint0")
            nc.sync.dma_start(wint0[:], d_win[0])
            wrst0 = wpool.tile([128, 1024], F32R, tag="wrs", name="wrst0")
            nc.sync.dma_start(wrst0[:], d_wrs[0])
            nc.gpsimd.dma_start(sbt[:], d_sb[:])
            # bulk loads on the ACT HWDGE queue (start-conv copies were moved
            # to DVE so these descriptor writes don't block anything)
            nc.scalar.dma_start(rbt[:], d_rb[:])
            for j in range(NCHUNK):
                nc.scalar.dma_start(
                    ctxs[:, j * TC : (j + 1) * TC], d_ctx[:, j * TC : (j + 1) * TC]
                )
            for i in range(L):
                nc.scalar.dma_start(
                    conds[:, i * TC : (i + 1) * TC], d_wcond[:, i * TC : (i + 1) * TC]
                )
            nc.scalar.dma_start(ends[:], d_wend[:])
            nc.scalar.dma_start(bendt[:], d_bend[:])

            # zero the halo columns of x once; updates only touch the center
            # (DMA'd zeros — DVE memset can't write float32r)
            nc.gpsimd.dma_start(x[:, 0:PAD], d_zeros[:, 0:PAD])
            nc.gpsimd.dma_start(x[:, PAD + T : XW + PAD], d_zeros[:])
            nc.gpsimd.dma_start(x[:, XW + PAD + T : 2 * XW], d_zeros[:, 0:PAD])

            # ---- start conv: x = start_w @ f0 + start_b ----
            for j in range(NCHUNK):
                ps_s = [
                    pspool.tile([128, TC], F32, tag="ps", name=f"ps_s{c}")
                    for c in range(2)
                ]
                for c in range(2):
                    nc.tensor.matmul(
                        ps_s[c][:],
                        lhsT=starts[:, c * 128 : (c + 1) * 128],
                        rhs=f0s[:, j * TC : (j + 1) * TC],
                        start=True,
                        stop=True,
                    )
                for c in range(2):
                    nc.vector.tensor_scalar_add(
                        x[:, c * XW + PAD + j * TC : c * XW + PAD + (j + 1) * TC],
                        ps_s[c][:],
                        sbt[:, c : c + 1],
                    )

            # ---- WN layers ----
            # Software-pipelined: the gating/res-skip/update work for chunk j
            # is emitted AFTER chunk j+1's in-conv matmuls, so (a) the PE
            # stream never waits on the ACT->DVE gating chain, and (b) the
            # x update for chunk j lands after chunk j+1's k=0 tap has read
            # the pre-update tail (correctness of the dilated conv halo).
            def chunk_tail(st_):
                i, j, ps_a, wrst, last = (
                    st_["i"],
                    st_["j"],
                    st_["ps_a"],
                    st_["wrst"],
                    st_["last"],
                )
                tt = tspool.tile([128, 1024], F32, tag="tt", name="tt")
                st = tspool.tile([128, 1024], F32, tag="st", name="st")
                for c in range(2):
                    nc.scalar.activation(
                        tt[:, c * TC : (c + 1) * TC], ps_a[c][:], AF.Tanh
                    )
                    nc.scalar.activation(
                        st[:, c * TC : (c + 1) * TC], ps_a[2 + c][:], AF.Sigmoid
                    )
                actst = apool.tile([128, 1024], F32R, tag="acts", name="actst")
                nc.vector.tensor_mul(actst[:], tt[:], st[:])

                nco = 2 if last else 4
                ps_r = [
                    pspool.tile([128, TC], F32, tag="ps", name=f"ps_r{c2}")
                    for c2 in range(nco)
                ]
                for c2 in range(nco):
                    for p in range(2):
                        nc.tensor.matmul(
                            ps_r[c2][:],
                            lhsT=wrst[
                                :,
                                p * TC + c2 * 128 : p * TC + (c2 + 1) * 128,
                            ],
                            rhs=actst[:, p * TC : (p + 1) * TC],
                            start=(p == 0),
                            stop=(p == 1),
                        )
                if not last:
                    for c2 in range(2):
                        xs = x[
                            :, c2 * XW + PAD + j * TC : c2 * XW + PAD + (j + 1) * TC
                        ]
                        nc.vector.scalar_tensor_tensor(
                            xs,
                            ps_r[c2][:],
                            rbt[:, i * 4 + c2 : i * 4 + c2 + 1],
                            xs,
                            ALU.add,
                            ALU.add,
                        )
                    for c2 in range(2, 4):
                        os_ = outacc[
                            :, (c2 - 2) * T + j * TC : (c2 - 2) * T + (j + 1) * TC
                        ]
                        if i == 0:
                            nc.vector.tensor_scalar_add(
                                os_,
                                ps_r[c2][:],
                                rbt[:, i * 4 + c2 : i * 4 + c2 + 1],
                            )
                        else:
                            nc.vector.scalar_tensor_tensor(
                                os_,
                                ps_r[c2][:],
                                rbt[:, i * 4 + c2 : i * 4 + c2 + 1],
                                os_,
                                ALU.add,
                                ALU.add,
                            )
                else:
                    for c2 in range(2):
                        os_ = outacc[:, c2 * T + j * TC : c2 * T + (j + 1) * TC]
                        nc.vector.scalar_tensor_tensor(
                            os_,
                            ps_r[c2][:],
                            rbt[:, i * 4 + c2 : i * 4 + c2 + 1],
                            os_,
                            ALU.add,
                            ALU.add,
                        )

            # ---- end conv + coupling (emitted per-chunk, interleaved) ----
            def end_chunk(j):
                ps_e = pspool.tile([8, TC], F32, tag="ps", name="ps_e")
                for p in range(2):
                    nc.tensor.matmul(
                        ps_e[:],
                        lhsT=ends[:, p * 8 : (p + 1) * 8],
                        rhs=outacc[:, p * T + j * TC : p * T + (j + 1) * TC],
                        start=(p == 0),
                        stop=(p == 1),
                    )
                esb = tailpool.tile([8, TC], F32, tag="esb", name="esb")
                nc.scalar.activation(esb[:], ps_e[:], AF.Identity, bias=bendt[:])
                nc.sync.dma_start(d_logs[:, j * TC : (j + 1) * TC], esb[0:N_IN, :])
                expt = tailpool.tile([N_IN, TC], F32, tag="expt", name="expt")
                nc.scalar.activation(expt[:], esb[0:N_IN, :], AF.Exp)
                bsh = tailpool.tile([N_IN, TC], F32, tag="bsh", name="bsh")
                nc.sync.dma_start(bsh[:], esb[N_IN : 2 * N_IN, :])
                f1c = tailpool.tile([N_IN, TC], F32, tag="f1c", name="f1c")
                nc.sync.dma_start(f1c[:], d_f1[:, j * TC : (j + 1) * TC])
                f1oc = tailpool.tile([N_IN, TC], F32, tag="f1oc", name="f1oc")
                nc.vector.tensor_mul(f1oc[:], expt[:], f1c[:])
                nc.vector.tensor_add(f1oc[:], f1oc[:], bsh[:])
                nc.sync.dma_start(d_f1o[:, j * TC : (j + 1) * TC], f1oc[:])


            prev = None  # pipeline state carried across chunks AND layers
            for i in range(L):
                d = DIL[i]
                last = i == L - 1
                if i == 0:
                    wint, wrst = wint0, wrst0
                else:
                    wint = wpool.tile([128, 3072], F32R, tag="win", name="wint")
                    nc.sync.dma_start(wint[:], d_win[i])
                    wrst = wpool.tile([128, 1024], F32R, tag="wrs", name="wrst")
                    nc.sync.dma_start(wrst[:], d_wrs[i])

                for j in range(NCHUNK):
                    ps_a = [
                        pspool.tile([128, TC], F32, tag="ps", name=f"ps_a{c}")
                        for c in range(4)
                    ]
                    for c in range(4):
                        po = ps_a[c][:]
                        for k in range(3):
                            off = PAD + j * TC + (k - 1) * d
                            for p in range(2):
                                nc.tensor.matmul(
                                    po,
                                    lhsT=wint[
                                        :,
                                        (k * 2 + p) * TC
                                        + c * 128 : (k * 2 + p) * TC
                                        + (c + 1) * 128,
                                    ],
                                    rhs=x[:, p * XW + off : p * XW + off + TC],
                                    start=(k == 0 and p == 0),
                                    stop=False,
                                )
                        nc.tensor.matmul(
                            po,
                            lhsT=conds[:, i * TC + c * 128 : i * TC + (c + 1) * 128],
                            rhs=ctxs[:, j * TC : (j + 1) * TC],
                            start=False,
                            stop=True,
                        )
                    if prev is not None:
                        chunk_tail(prev)
                        if prev["last"]:
                            end_chunk(prev["j"])
                    prev = {"i": i, "j": j, "ps_a": ps_a, "wrst": wrst, "last": last}
            chunk_tail(prev)
            end_chunk(prev["j"])
            prev = None

    split_multi_waits(nc)
    return nc


# ---------------------------------------------------------------------------
# Host-side weight/layout prep
# ---------------------------------------------------------------------------


def prep_shared(inputs):
    f32 = np.float32
    in_w = np.ascontiguousarray(inputs["in_w"], f32)  # [8, 512, 256, 3]
    w_in = np.ascontiguousarray(
        in_w.reshape(L, 2 * C, 2, 128, 3).transpose(0, 3, 4, 2, 1).reshape(L, 128, 3072)
    )
    rs_w = np.ascontiguousarray(inputs["rs_w"], f32)[..., 0]  # [8, 512, 256]
    w_rs = np.ascontiguousarray(
        rs_w.reshape(L, 2 * C, 2, 128).transpose(0, 3, 2, 1).reshape(L, 128, 1024)
    )
    cond_w = np.ascontiguousarray(inputs["cond_w"], f32)[..., 0]  # [4096, 80]
    ab = inputs["in_b"].reshape(-1) + inputs["cond_b"]  # [4096]
    w_cond = np.concatenate([cond_w.T, ab[None, :]], axis=0).astype(f32)  # [81, 4096]
    w_start = np.ascontiguousarray(inputs["start_w"][..., 0].T, f32)  # [4, 256]
    perm = [N_IN + k for k in range(N_IN)] + list(range(N_IN))  # [log_s..., b...]
    ew = inputs["end_w"][..., 0][perm]  # [8, 256]
    w_end = np.ascontiguousarray(
        ew.T.reshape(2, 128, 8).transpose(1, 0, 2).reshape(128, 16), f32
    )
    b_end = np.ascontiguousarray(inputs["end_b"][perm][:, None], f32)  # [8, 1]
    r_bias = np.ascontiguousarray(
        inputs["rs_b"].reshape(L, 4, 128).transpose(2, 0, 1).reshape(128, 4 * L), f32
    )
    s_bias = np.ascontiguousarray(inputs["start_b"].reshape(2, 128).T, f32)  # [128, 2]
    return {
        "w_in": w_in,
        "w_rs": w_rs,
        "w_cond": w_cond,
        "w_start": w_start,
        "w_end": w_end,
        "b_end": b_end,
        "r_bias": r_bias,
        "s_bias": s_bias,
    }


def _ensure_ntff_hook():
    """Register the axon NTFF profiling hook if the image's antenv lacks it."""
    import sys
    import types

    try:
        import antenv.axon_hooks  # noqa: F401

        return
    except ImportError:
        pass
    mod = types.ModuleType("antenv.axon_hooks")
    holder = [None]
    mod.set_axon_ntff_profile_hook = lambda h: holder.__setitem__(0, h)
    mod.get_axon_ntff_profile_hook = lambda: holder[0]
    sys.modules["antenv.axon_hooks"] = mod
    try:
        from trn_agent_boot.trn_boot import _ntff_profile_via_ctypes

        mod.set_axon_ntff_profile_hook(
            _ntff_profile_via_ctypes("/opt/axon/libaxon_pjrt.so")
        )
    except Exception:
        pass


_NC = None


def _get_program():
    global _NC
    if _NC is None:
        _NC = build_program()
    return _NC


LAST_RESULTS = None


def kernel(forecast, context, start_w, start_b, cond_w, cond_b,
           in_w, in_b, rs_w, rs_b, end_w, end_b, _trace=False):
    global LAST_RESULTS
    inputs = dict(
        forecast=np.asarray(forecast), context=np.asarray(context),
        start_w=np.asarray(start_w), start_b=np.asarray(start_b),
        cond_w=np.asarray(cond_w), cond_b=np.asarray(cond_b),
        in_w=np.asarray(in_w), in_b=np.asarray(in_b),
        rs_w=np.asarray(rs_w), rs_b=np.asarray(rs_b),
        end_w=np.asarray(end_w), end_b=np.asarray(end_b),
    )
    shared = prep_shared(inputs)
    shared["zeros"] = np.zeros((128, 2 * PAD), np.float32)
    forecast = np.ascontiguousarray(inputs["forecast"], np.float32)
    context = np.ascontiguousarray(inputs["context"], np.float32)
    ones_row = np.ones((1, T), np.float32)

    in_maps = []
    for b in range(B):
        m = dict(shared)
        m["f0"] = np.ascontiguousarray(forecast[b, :N_IN])
        m["f1"] = np.ascontiguousarray(forecast[b, N_IN:])
        m["ctxp"] = np.ascontiguousarray(
            np.concatenate([context[b], ones_row], axis=0)
        )
        in_maps.append(m)

    if _trace:
        _ensure_ntff_hook()
    nc = _get_program()
    try:
        res = bass_utils.run_bass_kernel_spmd(
            nc, in_maps, core_ids=list(range(B)), trace=_trace
        )
    except Exception:
        # transient device errors (e.g. NRT_EXEC_UNIT_UNRECOVERABLE after a
        # prior crashed run) usually clear on retry with a fresh NRT session
        import time

        time.sleep(2.0)
        res = bass_utils.run_bass_kernel_spmd(
            nc, in_maps, core_ids=list(range(B)), trace=_trace
        )
    LAST_RESULTS = res

    f1o = np.stack([res.results[b]["f1o"] for b in range(B)])  # [8, 4, T]
    logs = np.stack([res.results[b]["logs"] for b in range(B)])  # [8, 4, T]
    out_full = np.concatenate([forecast[:, :N_IN], f1o], axis=1)  # [8, 8, T]
    return out_full, logs


# revision 31
# speedup vs baseline: 1.0112x; 1.0009x over previous
"""Trainium2 Bass kernel for nn_AffineCoupling (WaveGlow-style WN coupling).

Sharding: data-parallel over batch — B=8 samples, one per NeuronCore. All
convs are per-sample so no cross-core communication is needed.

Per-core plan (T=4096, chunked into 8 x 512 columns):
  - x (residual, 256ch) lives in SBUF as [128, 2*(128+4096+128)] fp32 with
    zero pads so dilated-conv taps are plain shifted column reads.
  - per layer: a = in_conv(x) (K=3 dilated, 6 matmuls/chunk/coutblk)
    + cond matmul (K=81: context padded with a ones-row so the combined
    in_b+cond_b bias folds into the weight row) -> PSUM [128, 2048];
    acts = tanh(a[:256]) * sigmoid(a[256:]) (ACT+DVE); res_skip 1x1 conv
    (2 matmuls/chunk/coutblk) -> PSUM; x/out updates via DVE
    scalar_tensor_tensor (fuses the rs bias add).
  - end conv (K=256 -> 8ch, channels reordered to [log_s, b]) + ACT bias,
    exp via ACT, coupling math via DVE, outputs DMA'd out per chunk.

Matmuls run as float32r (same bits as fp32, ~11-bit-mantissa PE path at full
bf16-rate throughput) — all matmul-feeding tensors are declared float32r
end-to-end so no cast copies exist anywhere. Measured ~0.57 ms/core on HW
with max relative error ~4e-4 vs the fp32 reference.
"""

import copy

import numpy as np

import concourse.bass as bass
import concourse.mybir as mybir
import concourse.tile as tile
from concourse import bass_utils
from concourse.vector_clock import ScopedClock

F32 = mybir.dt.float32
F32R = mybir.dt.float32r
AF = mybir.ActivationFunctionType
ALU = mybir.AluOpType

N_IN = 4
N_CTX = 80
L = 8
C = 256
DIL = [1, 2, 4, 8, 16, 32, 64, 128]
B, T = 8, 4096
TC = 512
NCHUNK = T // TC
PAD = 128
XW = PAD + T + PAD  # per cin-block padded width

MM_DT = mybir.dt.float32r  # matmul operand dtype tag


# ---------------------------------------------------------------------------
# Workarounds for the walrus build in this environment: it rejects any
# instruction carrying more than one sync-wait. Split extra waits onto
# single-wait carrier instructions placed just before the owner (same
# engine, so engine program order preserves semantics).
# ---------------------------------------------------------------------------


def _patched_drain_and_barrier(self, tick_clock, wait_clock):
    nc = self.nc
    tmp = nc.sync.nop(nofuse=True)
    wait_clock.add_sem_waits(tmp.ins, ScopedClock({None: tick_clock.global_clock}))
    si = tmp.ins.sync_info
    waits = list(si.on_wait) if si is not None else []
    if waits:
        si.on_wait = [waits[0]]
        for w in waits[1:]:
            n = nc.sync.nop(nofuse=True)
            n.ins.sync_info = mybir.SyncInfo(on_wait=[w], on_update=[])
    nc.sync.drain()
    nc.all_engine_barrier()
    popped = nc._tile_sem_poison_stack.pop()
    assert popped is self._sem_poison
    nc.clear_and_free_semaphores(list(self.sems.allocated().values()))


tile.TileContext._drain_and_barrier = _patched_drain_and_barrier


def split_multi_waits(nc):
    template = None
    for f in nc.m.functions:
        for bb in f.blocks:
            for inst in bb.instructions:
                if type(inst).__name__ == "InstEventSemaphore":
                    template = inst
                    break
            if template is not None:
                break
        if template is not None:
            break
    assert template is not None
    ctr = 0
    for f in nc.m.functions:
        for bb in f.blocks:
            insts = bb.instructions
            if not any(
                i.sync_info is not None and len(i.sync_info.on_wait) > 1
                for i in insts
            ):
                continue
            new = []
            for inst in insts:
                si = inst.sync_info
                if si is not None and len(si.on_wait) > 1:
                    waits = list(si.on_wait)
                    for w in waits[:-1]:
                        c = copy.copy(template)
                        c.name = f"waitsplit-{ctr}"
                        ctr += 1
                        c.engine = inst.engine
                        c.sync_info = mybir.SyncInfo(on_wait=[w], on_update=[])
                        new.append(c)
                    si.on_wait = [waits[-1]]
                new.append(inst)
            bb.instructions = new
    return ctr


# ---------------------------------------------------------------------------
# Program builder
# ---------------------------------------------------------------------------


def build_program():
    nc = bass.Bass("TRN2", target_bir_lowering=False, debug=False, num_devices=B)

    d_f0 = nc.dram_tensor("f0", [N_IN, T], F32R, kind="ExternalInput").ap()
    d_f1 = nc.dram_tensor("f1", [N_IN, T], F32, kind="ExternalInput").ap()
    d_ctx = nc.dram_tensor("ctxp", [N_CTX + 1, T], F32R, kind="ExternalInput").ap()
    d_win = nc.dram_tensor("w_in", [L, 128, 3072], F32R, kind="ExternalInput").ap()
    d_wrs = nc.dram_tensor("w_rs", [L, 128, 1024], F32R, kind="ExternalInput").ap()
    d_wcond = nc.dram_tensor("w_cond", [N_CTX + 1, 4096], F32R, kind="ExternalInput").ap()
    d_wstart = nc.dram_tensor("w_start", [N_IN, 256], F32R, kind="ExternalInput").ap()
    d_wend = nc.dram_tensor("w_end", [128, 16], F32R, kind="ExternalInput").ap()
    d_bend = nc.dram_tensor("b_end", [8, 1], F32, kind="ExternalInput").ap()
    d_rb = nc.dram_tensor("r_bias", [128, 4 * L], F32, kind="ExternalInput").ap()
    d_sb = nc.dram_tensor("s_bias", [128, 2], F32, kind="ExternalInput").ap()
    d_zeros = nc.dram_tensor("zeros", [128, 2 * PAD], F32R, kind="ExternalInput").ap()

    d_f1o = nc.dram_tensor("f1o", [N_IN, T], F32, kind="ExternalOutput").ap()
    d_logs = nc.dram_tensor("logs", [N_IN, T], F32, kind="ExternalOutput").ap()

    with tile.TileContext(nc) as tc:
        from contextlib import ExitStack

        ctx = ExitStack()
        with ctx:
            const = ctx.enter_context(tc.tile_pool(name="const", bufs=1))
            wpool = ctx.enter_context(tc.tile_pool(name="wpool", bufs=2))
            tspool = ctx.enter_context(tc.tile_pool(name="tspool", bufs=2))
            apool = ctx.enter_context(tc.tile_pool(name="apool", bufs=2))
            tailpool = ctx.enter_context(tc.tile_pool(name="tailpool", bufs=1))
            pspool = ctx.enter_context(tc.tile_pool(name="pspool", bufs=8, space="PSUM"))

            x = const.tile([128, 2 * XW], F32R, name="x")
            outacc = const.tile([128, 2 * T], F32R, name="outacc")
            ctxs = const.tile([N_CTX + 1, T], F32R, name="ctxs")
            f0s = const.tile([N_IN, T], F32R, name="f0s")
            conds = const.tile([N_CTX + 1, 4096], F32R, name="conds")
            starts = const.tile([N_IN, 256], F32R, name="starts")
            ends = const.tile([128, 16], F32R, name="ends")
            bendt = const.tile([8, 1], F32, name="bendt")
            rbt = const.tile([128, 4 * L], F32, name="rbt")
            sbt = const.tile([128, 2], F32, name="sbt")
            f1s = const.tile([N_IN, T], F32, name="f1s")

            # PE warm-up: the HAM clock gate needs ~3.4us of sustained matmul
            # activity to lift the PE from 1.2 to 2.4 GHz. Burn the initial
            # DMA-wait window on dummy matmuls over a zeroed scratch tile so
            # the real matmuls start warm.
            scratch = const.tile([128, 64], F32, name="scratch")
            nc.vector.memset(scratch[:], 0.0)
            for w in range(3):
                ps_w = pspool.tile([64, 64], F32, tag="ps", name="ps_w")
                for _ in range(8):
                    nc.tensor.matmul(
                        ps_w[:], lhsT=scratch[:, 0:64], rhs=scratch[:], start=True, stop=True
                    )

            # critical-path loads on the SP queue (start conv inputs first)
            nc.sync.dma_start(starts[:], d_wstart[:])
            nc.sync.dma_start(f0s[:], d_f0[:])
            # layer-0 weights early on the SP queue so the first in-conv
            # matmuls don't wait behind anything else
            wint0 = wpool.tile([128, 3072], F32R, tag="win", name="wint0")
            nc.sync.dma_start(wint0[:], d_win[0])
            wrst0 = wpool.tile([128, 1024], F32R, tag="wrs", name="wrst0")
            nc.sync.dma_start(wrst0[:], d_wrs[0])
            nc.gpsimd.dma_start(sbt[:], d_sb[:])
            # bulk loads on the ACT HWDGE queue (start-conv copies were moved
            # to DVE so these descriptor writes don't block anything)
            nc.scalar.dma_start(rbt[:], d_rb[:])
            for j in range(NCHUNK):
                nc.scalar.dma_start(
                    ctxs[:, j * TC : (j + 1) * TC], d_ctx[:, j * TC : (j + 1) * TC]
                )
            for i in range(L):
                nc.scalar.dma_start(
                    conds[:, i * TC : (i + 1) * TC], d_wcond[:, i * TC : (i + 1) * TC]
                )
            nc.scalar.dma_start(f1s[:], d_f1[:])
            nc.scalar.dma_start(ends[:], d_wend[:])
            nc.scalar.dma_start(bendt[:], d_bend[:])

            # zero the halo columns of x once; updates only touch the center
            # (DMA'd zeros — DVE memset can't write float32r)
            nc.gpsimd.dma_start(x[:, 0:PAD], d_zeros[:, 0:PAD])
            nc.gpsimd.dma_start(x[:, PAD + T : XW + PAD], d_zeros[:])
            nc.gpsimd.dma_start(x[:, XW + PAD + T : 2 * XW], d_zeros[:, 0:PAD])

            # ---- start conv: x = start_w @ f0 + start_b ----
            for j in range(NCHUNK):
                ps_s = [
                    pspool.tile([128, TC], F32, tag="ps", name=f"ps_s{c}")
                    for c in range(2)
                ]
                for c in range(2):
                    nc.tensor.matmul(
                        ps_s[c][:],
                        lhsT=starts[:, c * 128 : (c + 1) * 128],
                        rhs=f0s[:, j * TC : (j + 1) * TC],
                        start=True,
                        stop=True,
                    )
                for c in range(2):
                    nc.vector.tensor_scalar_add(
                        x[:, c * XW + PAD + j * TC : c * XW + PAD + (j + 1) * TC],
                        ps_s[c][:],
                        sbt[:, c : c + 1],
                    )

            # ---- WN layers ----
            # Software-pipelined: the gating/res-skip/update work for chunk j
            # is emitted AFTER chunk j+1's in-conv matmuls, so (a) the PE
            # stream never waits on the ACT->DVE gating chain, and (b) the
            # x update for chunk j lands after chunk j+1's k=0 tap has read
            # the pre-update tail (correctness of the dilated conv halo).
            def chunk_tail(st_):
                i, j, ps_a, wrst, last = (
                    st_["i"],
                    st_["j"],
                    st_["ps_a"],
                    st_["wrst"],
                    st_["last"],
                )
                tt = tspool.tile([128, 1024], F32, tag="tt", name="tt")
                st = tspool.tile([128, 1024], F32, tag="st", name="st")
                for c in range(2):
                    nc.scalar.activation(
                        tt[:, c * TC : (c + 1) * TC], ps_a[c][:], AF.Tanh
                    )
                    nc.scalar.activation(
                        st[:, c * TC : (c + 1) * TC], ps_a[2 + c][:], AF.Sigmoid
                    )
                actst = apool.tile([128, 1024], F32R, tag="acts", name="actst")
                nc.vector.tensor_mul(actst[:], tt[:], st[:])

                nco = 2 if last else 4
                ps_r = [
                    pspool.tile([128, TC], F32, tag="ps", name=f"ps_r{c2}")
                    for c2 in range(nco)
                ]
                for c2 in range(nco):
                    for p in range(2):
                        nc.tensor.matmul(
                            ps_r[c2][:],
                            lhsT=wrst[
                                :,
                                p * TC + c2 * 128 : p * TC + (c2 + 1) * 128,
                            ],
                            rhs=actst[:, p * TC : (p + 1) * TC],
                            start=(p == 0),
                            stop=(p == 1),
                        )
                if not last:
                    for c2 in range(2):
                        xs = x[
                            :, c2 * XW + PAD + j * TC : c2 * XW + PAD + (j + 1) * TC
                        ]
                        nc.vector.scalar_tensor_tensor(
                            xs,
                            ps_r[c2][:],
                            rbt[:, i * 4 + c2 : i * 4 + c2 + 1],
                            xs,
                            ALU.add,
                            ALU.add,
                        )
                    for c2 in range(2, 4):
                        os_ = outacc[
                            :, (c2 - 2) * T + j * TC : (c2 - 2) * T + (j + 1) * TC
                        ]
                        if i == 0:
                            nc.vector.tensor_scalar_add(
                                os_,
                                ps_r[c2][:],
                                rbt[:, i * 4 + c2 : i * 4 + c2 + 1],
                            )
                        else:
                            nc.vector.scalar_tensor_tensor(
                                os_,
                                ps_r[c2][:],
                                rbt[:, i * 4 + c2 : i * 4 + c2 + 1],
                                os_,
                                ALU.add,
                                ALU.add,
                            )
                else:
                    for c2 in range(2):
                        os_ = outacc[:, c2 * T + j * TC : c2 * T + (j + 1) * TC]
                        nc.vector.scalar_tensor_tensor(
                            os_,
                            ps_r[c2][:],
                            rbt[:, i * 4 + c2 : i * 4 + c2 + 1],
                            os_,
                            ALU.add,
                            ALU.add,
                        )

            # ---- end conv + coupling (emitted per-chunk, interleaved) ----
            def end_chunk(j):
                ps_e = pspool.tile([8, TC], F32, tag="ps", name="ps_e")
                for p in range(2):
                    nc.tensor.matmul(
                        ps_e[:],
                        lhsT=ends[:, p * 8 : (p + 1) * 8],
                        rhs=outacc[:, p * T + j * TC : p * T + (j + 1) * TC],
                        start=(p == 0),
                        stop=(p == 1),
                    )
                esb = tailpool.tile([8, TC], F32, tag="esb", name="esb")
                nc.scalar.activation(esb[:], ps_e[:], AF.Identity, bias=bendt[:])
                nc.sync.dma_start(d_logs[:, j * TC : (j + 1) * TC], esb[0:N_IN, :])
                expt = tailpool.tile([N_IN, TC], F32, tag="expt", name="expt")
                nc.scalar.activation(expt[:], esb[0:N_IN, :], AF.Exp)
                bsh = tailpool.tile([N_IN, TC], F32, tag="bsh", name="bsh")
                nc.sync.dma_start(bsh[:], esb[N_IN : 2 * N_IN, :])
                f1oc = tailpool.tile([N_IN, TC], F32, tag="f1oc", name="f1oc")
                nc.vector.tensor_mul(f1oc[:], expt[:], f1s[:, j * TC : (j + 1) * TC])
                nc.vector.tensor_add(f1oc[:], f1oc[:], bsh[:])
                nc.sync.dma_start(d_f1o[:, j * TC : (j + 1) * TC], f1oc[:])


            prev = None  # pipeline state carried across chunks AND layers
            for i in range(L):
                d = DIL[i]
                last = i == L - 1
                if i == 0:
                    wint, wrst = wint0, wrst0
                else:
                    wint = wpool.tile([128, 3072], F32R, tag="win", name="wint")
                    nc.sync.dma_start(wint[:], d_win[i])
                    wrst = wpool.tile([128, 1024], F32R, tag="wrs", name="wrst")
                    nc.sync.dma_start(wrst[:], d_wrs[i])

                for j in range(NCHUNK):
                    ps_a = [
                        pspool.tile([128, TC], F32, tag="ps", name=f"ps_a{c}")
                        for c in range(4)
                    ]
                    for c in range(4):
                        po = ps_a[c][:]
                        for k in range(3):
                            off = PAD + j * TC + (k - 1) * d
                            for p in range(2):
                                nc.tensor.matmul(
                                    po,
                                    lhsT=wint[
                                        :,
                                        (k * 2 + p) * TC
                                        + c * 128 : (k * 2 + p) * TC
                                        + (c + 1) * 128,
                                    ],
                                    rhs=x[:, p * XW + off : p * XW + off + TC],
                                    start=(k == 0 and p == 0),
                                    stop=False,
                                )
                        nc.tensor.matmul(
                            po,
                            lhsT=conds[:, i * TC + c * 128 : i * TC + (c + 1) * 128],
                            rhs=ctxs[:, j * TC : (j + 1) * TC],
                            start=False,
                            stop=True,
                        )
                    if prev is not None:
                        chunk_tail(prev)
                        if prev["last"]:
                            end_chunk(prev["j"])
                    prev = {"i": i, "j": j, "ps_a": ps_a, "wrst": wrst, "last": last}
            chunk_tail(prev)
            end_chunk(prev["j"])
            prev = None

    split_multi_waits(nc)
    return nc


# ---------------------------------------------------------------------------
# Host-side weight/layout prep
# ---------------------------------------------------------------------------


def prep_shared(inputs):
    f32 = np.float32
    in_w = np.ascontiguousarray(inputs["in_w"], f32)  # [8, 512, 256, 3]
    w_in = np.ascontiguousarray(
        in_w.reshape(L, 2 * C, 2, 128, 3).transpose(0, 3, 4, 2, 1).reshape(L, 128, 3072)
    )
    rs_w = np.ascontiguousarray(inputs["rs_w"], f32)[..., 0]  # [8, 512, 256]
    w_rs = np.ascontiguousarray(
        rs_w.reshape(L, 2 * C, 2, 128).transpose(0, 3, 2, 1).reshape(L, 128, 1024)
    )
    cond_w = np.ascontiguousarray(inputs["cond_w"], f32)[..., 0]  # [4096, 80]
    ab = inputs["in_b"].reshape(-1) + inputs["cond_b"]  # [4096]
    w_cond = np.concatenate([cond_w.T, ab[None, :]], axis=0).astype(f32)  # [81, 4096]
    w_start = np.ascontiguousarray(inputs["start_w"][..., 0].T, f32)  # [4, 256]
    perm = [N_IN + k for k in range(N_IN)] + list(range(N_IN))  # [log_s..., b...]
    ew = inputs["end_w"][..., 0][perm]  # [8, 256]
    w_end = np.ascontiguousarray(
        ew.T.reshape(2, 128, 8).transpose(1, 0, 2).reshape(128, 16), f32
    )
    b_end = np.ascontiguousarray(inputs["end_b"][perm][:, None], f32)  # [8, 1]
    r_bias = np.ascontiguousarray(
        inputs["rs_b"].reshape(L, 4, 128).transpose(2, 0, 1).reshape(128, 4 * L), f32
    )
    s_bias = np.ascontiguousarray(inputs["start_b"].reshape(2, 128).T, f32)  # [128, 2]
    return {
        "w_in": w_in,
        "w_rs": w_rs,
        "w_cond": w_cond,
        "w_start": w_start,
        "w_end": w_end,
        "b_end": b_end,
        "r_bias": r_bias,
        "s_bias": s_bias,
    }


def _ensure_ntff_hook():
    """Register the axon NTFF profiling hook if the image's antenv lacks it."""
    import sys
    import types

    try:
        import antenv.axon_hooks  # noqa: F401

        return
    except ImportError:
        pass
    mod = types.ModuleType("antenv.axon_hooks")
    holder = [None]
    mod.set_axon_ntff_profile_hook = lambda h: holder.__setitem__(0, h)
    mod.get_axon_ntff_profile_hook = lambda: holder[0]
    sys.modules["antenv.axon_hooks"] = mod
    try:
        from trn_agent_boot.trn_boot import _ntff_profile_via_ctypes

        mod.set_axon_ntff_profile_hook(
            _ntff_profile_via_ctypes("/opt/axon/libaxon_pjrt.so")
        )
    except Exception:
        pass


_NC = None


def _get_program():
    global _NC
    if _NC is None:
        _NC = build_program()
    return _NC


LAST_RESULTS = None


def kernel(forecast, context, start_w, start_b, cond_w, cond_b,
           in_w, in_b, rs_w, rs_b, end_w, end_b, _trace=False):
    global LAST_RESULTS
    inputs = dict(
        forecast=np.asarray(forecast), context=np.asarray(context),
        start_w=np.asarray(start_w), start_b=np.asarray(start_b),
        cond_w=np.asarray(cond_w), cond_b=np.asarray(cond_b),
        in_w=np.asarray(in_w), in_b=np.asarray(in_b),
        rs_w=np.asarray(rs_w), rs_b=np.asarray(rs_b),
        end_w=np.asarray(end_w), end_b=np.asarray(end_b),
    )
    shared = prep_shared(inputs)
    shared["zeros"] = np.zeros((128, 2 * PAD), np.float32)
    forecast = np.ascontiguousarray(inputs["forecast"], np.float32)
    context = np.ascontiguousarray(inputs["context"], np.float32)
    ones_row = np.ones((1, T), np.float32)

    in_maps = []
    for b in range(B):
        m = dict(shared)
        m["f0"] = np.ascontiguousarray(forecast[b, :N_IN])
        m["f1"] = np.ascontiguousarray(forecast[b, N_IN:])
        m["ctxp"] = np.ascontiguousarray(
            np.concatenate([context[b], ones_row], axis=0)
        )
        in_maps.append(m)

    if _trace:
        _ensure_ntff_hook()
    nc = _get_program()
    try:
        res = bass_utils.run_bass_kernel_spmd(
            nc, in_maps, core_ids=list(range(B)), trace=_trace
        )
    except Exception:
        # transient device errors (e.g. NRT_EXEC_UNIT_UNRECOVERABLE after a
        # prior crashed run) usually clear on retry with a fresh NRT session
        import time

        time.sleep(2.0)
        res = bass_utils.run_bass_kernel_spmd(
            nc, in_maps, core_ids=list(range(B)), trace=_trace
        )
    LAST_RESULTS = res

    f1o = np.stack([res.results[b]["f1o"] for b in range(B)])  # [8, 4, T]
    logs = np.stack([res.results[b]["logs"] for b in range(B)])  # [8, 4, T]
    out_full = np.concatenate([forecast[:, :N_IN], f1o], axis=1)  # [8, 8, T]
    return out_full, logs
